# revision 3
# baseline (speedup 1.0000x reference)
"""Causal self-attention (per-head A projections) on 8 TRN2 NeuronCores.

Shapes: h [B=2, T=2048, d=64] f32, A [H=8, d, d] f32.
  q = h @ A[i]; scores = q @ h^T (causal); out_i = softmax(scores) @ h.
Sharding: one head per core (embarrassingly parallel, no collectives).
Each core receives the full h and its own A[i]; host concatenates heads.

v2: all score matmuls in fp16 (1 cyc/col on the PE at any moving width,
vs ~3.3 cyc/col measured for f32r), bf16 only after the exp (pT/AV need
bf16's exponent range). Masks use -60000 (fp16-representable; exp -> 0).

Chunk-pipelined two-pass softmax, emitted as interleaved units:

  PRE(b,c): PE-transpose h chunk (fp16) into half 0 of a 2-bank PSUM tile
      and qT = A-matmul into half 1; ACT/DVE copy them out into per-batch
      merged hc/qc [65, T] fp16 tiles; Pool casts hs16 (fp16, transpose
      src) and DVE derives hsb (bf16, AV rhs).
  S(b,g): stats row-max for t-tiles 4g..4g+3.  fp16 score matmuls over
      column windows of hc; -60000 upper-triangle accumulated via identity
      matmul on the diagonal window.  Window PAIRS are laid out
      contiguously across the two banks of one PSUM tile so ONE DVE
      reduce_max consumes each pair; negated -m (fp16) lands in row 64 of
      qc via a [128,1] SBUF DMA reshape.
  P(b,c): pass2 scoresT tiles [s,t] (fp16, K=65 so the -m row folds the
      subtraction into the matmul), diagonal tiles at exact causal width
      and -60000 on the acausal diagonal part accumulated by an identity
      matmul (PE) before ACT's exp -> pT bf16; AV accumulates
      NATURAL-layout out tiles [t, d+1] in PSUM with lhsT=pT column
      slices and rhs=hsb (ones column = softmax denominator l).
      NOTE: matmul start=True clears has_written for the whole PSUM bank,
      so only the first AV matmul into each oT bank sets it.  Finalize:
      DVE reciprocal of the four l columns + per-tile scale (ACT/DVE),
      one output DMA per chunk.
"""

import os
import sys

for _p in ("/opt/trn_rl_repo",):
    if _p not in sys.path:
        sys.path.insert(0, _p)

import numpy as np
from contextlib import ExitStack

import concourse.bass as bass
import concourse.tile as tile
from concourse import bacc, mybir
from concourse.masks import make_identity
from concourse.bass_utils import run_bass_kernel_spmd

B, T, D, H = 2, 2048, 64, 8
P = 128                # square tile size (t and s)
NT = T // P            # 16 tiles along t/s
CH = 512               # chunk width (PSUM bank)
NCH = T // CH          # 4 chunks
NEG = -60000.0         # fp16-representable -inf stand-in (exp -> 0)

f32 = mybir.dt.float32
fp16 = mybir.dt.float16
bf16 = mybir.dt.bfloat16


def _stat_windows(i):
    """Column windows covering the causal region [0, 128*(i+1)) for t-tile i.

    First window is rem wide (s_end % 512); remaining windows are 512 wide,
    end-aligned so the last one lands exactly on the causal boundary (where
    the mask goes).
    """
    s_end = (i + 1) * P
    rem = s_end % CH
    wins = []
    start = 0
    if rem:
        wins.append((0, rem))
        start = rem
    wins.extend(
        (start + k * CH, start + (k + 1) * CH) for k in range((s_end - start) // CH)
    )
    return wins


def _build(ctx: ExitStack, tc: "tile.TileContext", h_ext, A_ext, out_ext):
    nc = tc.nc

    consts = ctx.enter_context(tc.tile_pool(name="consts", bufs=1))
    hpool = ctx.enter_context(tc.tile_pool(name="hpool", bufs=2))
    qpool = ctx.enter_context(tc.tile_pool(name="qpool", bufs=2))
    ppool = ctx.enter_context(tc.tile_pool(name="ppool", bufs=6))
    spool = ctx.enter_context(tc.tile_pool(name="spool", bufs=4))
    opool = ctx.enter_context(tc.tile_pool(name="opool", bufs=2))
    # PSUM (8 banks): stats/pre 2-bank tiles x2 + pass2 x2 + out x2.
    ps_st = ctx.enter_context(tc.tile_pool(name="ps_st", bufs=2, space="PSUM"))
    ps_p2 = ctx.enter_context(tc.tile_pool(name="ps_p2", bufs=3, space="PSUM"))
    ps_out = ctx.enter_context(tc.tile_pool(name="ps_out", bufs=1, space="PSUM"))

    # ---- constants ----
    identh = consts.tile([P, P], fp16)
    make_identity(nc, identh)

    # umask[t, s] = NEG where s > t else 0 (stats-pass diagonal tile).
    umask = consts.tile([P, P], fp16)
    nc.gpsimd.memset(umask, 0.0)
    nc.gpsimd.affine_select(
        out=umask, in_=umask, compare_op=mybir.AluOpType.is_ge, fill=NEG,
        base=0, channel_multiplier=1, pattern=[[-1, P]],
    )
    # lmask[s, t] = NEG where t < s else 0 (pass-2 diagonal tile, scoresT).
    lmask = consts.tile([P, P], fp16)
    nc.gpsimd.memset(lmask, 0.0)
    nc.gpsimd.affine_select(
        out=lmask, in_=lmask, compare_op=mybir.AluOpType.is_ge, fill=NEG,
        base=0, channel_multiplier=-1, pattern=[[1, P]],
    )
    # A for this core's head: [d, e] natural layout (d on partitions), fp16.
    Asb32 = consts.tile([D, D], f32)
    nc.sync.dma_start(out=Asb32, in_=A_ext)
    Asb = consts.tile([D, D], fp16)
    nc.scalar.copy(Asb, Asb32)

    # ---- bulk input loads (both batches up front) ----
    hs32s, hs16s, hsbs, hcs, qcs = [], [], [], [], []
    for b in range(B):
        hs32 = hpool.tile([P, NT, D + 1], f32, tag="hs32", name=f"hs32_{b}")
        h_re = h_ext[b].rearrange("(j p) d -> p j d", p=P)
        if b == 0:
            # fine-grained first-chunk loads so the first transpose starts
            # as early as possible (shorter pipeline ramp)
            for j in range(4):
                nc.sync.dma_start(out=hs32[:, j, 0:D], in_=h_re[:, j, :])
            nc.sync.dma_start(out=hs32[:, 4:NT, 0:D], in_=h_re[:, 4:NT, :])
        else:
            for c in range(NCH):
                nc.sync.dma_start(
                    out=hs32[:, 4 * c : 4 * c + 4, 0:D],
                    in_=h_re[:, 4 * c : 4 * c + 4, :],
                )
        nc.gpsimd.memset(hs32[:, :, D : D + 1], 1.0)
        hs32s.append(hs32)
        hs16s.append(hpool.tile([P, NT, D + 1], fp16, tag="hs16", name=f"hs16_{b}"))
        hsbs.append(hpool.tile([P, NT, D + 1], bf16, tag="hsb", name=f"hsb_{b}"))
        hcs.append(hpool.tile([D + 1, T], fp16, tag="hc", name=f"hc_{b}"))
        qcs.append(qpool.tile([D + 1, T], fp16, tag="qc", name=f"qc_{b}"))

    def emit_pre(b, c, copies_act=False):
        """Transpose h chunk + qT chunk through one 2-bank PSUM tile."""
        lo = c * CH
        cp = nc.scalar.copy if copies_act else nc.vector.tensor_copy
        cp2 = nc.scalar.copy if b == 1 else cp  # qc off DVE in the busy middle
        # fp16 casts for this chunk: transpose source (Pool) + AV rhs (DVE).
        nc.gpsimd.tensor_copy(
            hs16s[b][:, 4 * c : 4 * c + 4, :], hs32s[b][:, 4 * c : 4 * c + 4, :]
        )
        nc.vector.tensor_copy(
            hsbs[b][:, 4 * c : 4 * c + 4, :], hs16s[b][:, 4 * c : 4 * c + 4, :]
        )
        stile = ps_st.tile([P, 2 * CH], f32, tag="st", name=f"pre_{b}_{c}")
        # fp16 transpose output: PE writes packed fp16 pairs into PSUM cells;
        # view the first 256 f32 cells of bank 0 as [65, 512] fp16.
        pt = stile[0 : D + 1, 0 : CH // 2].bitcast(fp16)
        for r in range(4):
            j = 4 * c + r
            nc.tensor.transpose(pt[:, r * P : (r + 1) * P], hs16s[b][:, j, :], identh)
        hc = hcs[b][:, lo : lo + CH]
        cp(hc, pt)

        pq = stile[0:D, CH : 2 * CH]
        nc.tensor.matmul(pq, lhsT=Asb, rhs=hc[0:D, :], start=True, stop=True)
        cp2(qcs[b][0:D, lo : lo + CH], pq)

    def emit_stats(b, g):
        """Row-max for t-tiles 4g..4g+3 -> -m into qc row 64."""
        for i in range(4 * g, 4 * g + 4):
            s_end = (i + 1) * P
            wins = _stat_windows(i)
            single = len(wins) <= 2  # one packed reduce -> write -m direct
            mxp = spool.tile([P, 2], f32, tag="mxp")
            negm = spool.tile([P, 1], fp16, tag="negm")
            lhs_q = qcs[b][0:D, i * P : (i + 1) * P]
            nred = 0
            for p0 in range(0, len(wins), 2):
                pair = wins[p0 : p0 + 2]
                flat = ps_st.tile([P, 2 * CH], f32, tag="st")
                # Lay the pair out so one DVE op consumes it: a partial-width
                # leading window is packed right against the bank boundary
                # (single contiguous reduce), a (512, 512) pair fills both
                # banks.
                if len(pair) == 2:
                    (lo0, hi0), (lo1, hi1) = pair
                    w0, w1 = hi0 - lo0, hi1 - lo1
                    spans = [(lo0, hi0, CH - w0, CH), (lo1, hi1, CH, CH + w1)]
                else:
                    (lo0, hi0) = pair[0]
                    w0 = hi0 - lo0
                    spans = [(lo0, hi0, 0, w0)]
                for lo, hi, a, bnd in spans:
                    nc.tensor.matmul(
                        flat[:, a:bnd], lhsT=lhs_q, rhs=hcs[b][0:D, lo:hi],
                        start=True, stop=not hi == s_end, skip_group_check=True,
                    )
                    if hi == s_end:
                        nc.tensor.matmul(
                            flat[:, bnd - P : bnd], lhsT=identh, rhs=umask,
                            start=False, stop=True, skip_group_check=True,
                        )
                # One contiguous DVE reduce per pair (packed across the bank
                # boundary); DVE may read only ONE PSUM operand per op.
                a0 = spans[0][2]
                a1 = spans[-1][3]
                if single:
                    nc.vector.reduce_max(
                        negm, flat[:, a0:a1], axis=mybir.AxisListType.X,
                        negate=True,
                    )
                else:
                    nc.vector.reduce_max(
                        mxp[:, nred : nred + 1], flat[:, a0:a1],
                        axis=mybir.AxisListType.X,
                    )
                nred += 1
            # Second-level max + negate in one DVE op.
            if not single:
                nc.vector.reduce_max(
                    negm, mxp[:, 0:nred], axis=mybir.AxisListType.X, negate=True
                )
            # Partition-column -> free-row reshape via a tiny SBUF->SBUF DMA.
            nc.sync.dma_start(
                out=qcs[b][D : D + 1, i * P : (i + 1) * P], in_=negm
            )

    class P2Chunk:
        """Pass2 scoresT + exp + natural-layout AV + finalize for one chunk.

        Split into per-j steps so two chunks can be emitted zipper-style
        (alternating j-steps) to pipeline the ACT-bound kernel tail.
        """

        def __init__(self, b, c, scale_dve=False, scale_act=False):
            self.b, self.c, self.scale_dve = b, c, scale_dve
            self.scale_act = scale_act
            self.oT = ps_out.tile([P, 4, D + 8], f32, tag="oT", name=f"oT_{b}_{c}")
            self.osb = opool.tile([P, 4, D], f32, tag="osb", name=f"osb_{b}_{c}")
            self.av_queue = []
            self.nsteps = 4 * c + 4

        def flush_av(self, limit):
            while len(self.av_queue) > limit:
                jq, pTq = self.av_queue.pop(0)
                for k in range(4):
                    i = 4 * self.c + k
                    if jq > i:
                        continue
                    # start=True clears has_written for the WHOLE bank, so
                    # only the very first matmul into this oT bank may set
                    # it; later first-writes hit cleared bits and overwrite.
                    nc.tensor.matmul(
                        self.oT[:, k, 0 : D + 1], lhsT=pTq[:, k * P : (k + 1) * P],
                        rhs=hsbs[self.b][:, jq, :],
                        start=(jq == 0 and k == 0), stop=(jq == i),
                        skip_group_check=True,
                    )

        def step(self, j):
            b, c = self.b, self.c
            r = j - 4 * c  # >= 0 on diagonal tiles
            diag = r >= 0
            wm = CH if not diag else CH - P * r  # causal width, exact
            ws = CH - wm
            toff = 0 if not diag else P * r  # first causal t column
            p2 = ps_p2.tile([P, CH], f32, tag="p2")
            nc.tensor.matmul(
                p2[:, ws:CH],
                lhsT=hcs[b][:, j * P : (j + 1) * P],
                rhs=qcs[b][:, c * CH + ws : (c + 1) * CH],
                start=True, stop=not diag, skip_group_check=True,
            )
            if diag:
                # -60000 on the acausal diagonal part, accumulated in PSUM by
                # the PE (keeps Pool out of the exp->AV dependency chain).
                nc.tensor.matmul(
                    p2[:, toff : toff + P], lhsT=identh, rhs=lmask,
                    start=False, stop=True, skip_group_check=True,
                )
            pT = ppool.tile([P, CH], bf16, tag="pT")
            nc.scalar.activation(
                pT[:, toff:CH], p2[:, toff:CH], mybir.ActivationFunctionType.Exp
            )
            self.av_queue.append((j, pT))
            self.flush_av(2)

        def finish(self):
            self.flush_av(0)
            # Finalize: strided reciprocal over the 4 l columns, then scale.
            oT, osb = self.oT, self.osb
            rl = spool.tile([P, 4], f32, tag="rl")
            for k in range(4):
                nc.vector.reciprocal(rl[:, k : k + 1], oT[:, k, D : D + 1])
            for k in range(4):
                if not self.scale_act and (self.scale_dve or k % 2 == 0):
                    nc.vector.tensor_scalar_mul(
                        osb[:, k, :], oT[:, k, 0:D], rl[:, k : k + 1]
                    )
                else:
                    nc.scalar.activation(
                        osb[:, k, :], oT[:, k, 0:D],
                        mybir.ActivationFunctionType.Copy, scale=rl[:, k : k + 1],
                    )
            nc.sync.dma_start(
                out=out_ext[self.b, self.c * CH : (self.c + 1) * CH, :].rearrange(
                    "(j p) d -> p j d", p=P
                ),
                in_=osb,
            )

    def emit_p2(b, c, scale_dve=False, scale_act=False):
        st = P2Chunk(b, c, scale_dve, scale_act)
        for j in range(st.nsteps):
            st.step(j)
        st.finish()

    def emit_p2_pair(b, c_lo, c_hi):
        """Zipper two chunks' j-loops so exp pipelines through the tail."""
        lo = P2Chunk(b, c_lo, scale_dve=True)
        hi = P2Chunk(b, c_hi, scale_dve=True)
        for j in range(hi.nsteps):
            hi.step(j)
            if j < lo.nsteps:
                lo.step(j)
            if j == lo.nsteps - 1:
                lo.finish()
        hi.finish()

    # ---- unit schedule ----
    # Batch 0 ascends (stats can start after one PRE chunk -> early DVE
    # ramp); batch 1 descends (heaviest stats/exp units mid-kernel, the
    # post-stats tail is only the two smallest chunks, zippered).
    emit_pre(0, 0)
    emit_stats(0, 0)
    emit_pre(1, 0)
    emit_pre(0, 1, copies_act=True)
    emit_stats(0, 1)
    emit_p2(0, 0, scale_act=True)
    emit_pre(0, 2, copies_act=True)
    emit_stats(0, 2)
    emit_p2(0, 1, scale_act=True)
    emit_pre(0, 3)
    emit_pre(1, 1)
    emit_stats(0, 3)
    emit_p2(0, 2)
    emit_pre(1, 2)
    emit_pre(1, 3)
    emit_stats(1, 3)
    emit_p2(0, 3)
    emit_p2(1, 3, scale_dve=True)
    emit_stats(1, 2)
    emit_p2(1, 2, scale_dve=True)
    emit_stats(1, 1)
    emit_p2(1, 1, scale_dve=True)
    emit_stats(1, 0)
    emit_p2(1, 0, scale_dve=True)


_cache = {}


def _get_nc():
    if "nc" not in _cache:
        nc = bacc.Bacc(
            "TRN2", target_bir_lowering=False, debug=False, num_devices=H
        )
        h_ext = nc.dram_tensor("h", [B, T, D], f32, kind="ExternalInput").ap()
        A_ext = nc.dram_tensor("A", [D, D], f32, kind="ExternalInput").ap()
        out_ext = nc.dram_tensor("out", [B, T, D], f32, kind="ExternalOutput").ap()
        with tile.TileContext(nc) as tc:
            with ExitStack() as ctx:
                _build(ctx, tc, h_ext, A_ext, out_ext)
        nc.compile()
        _cache["nc"] = nc
    return _cache["nc"]


def run(h, A, **kw):
    """Run on hardware; returns (full output [B,T,H*D], BassKernelResults)."""
    nc = _get_nc()
    h = np.ascontiguousarray(h, dtype=np.float32)
    A = np.ascontiguousarray(A, dtype=np.float32)
    in_maps = [{"h": h, "A": np.ascontiguousarray(A[i])} for i in range(H)]
    res = run_bass_kernel_spmd(nc, in_maps, core_ids=list(range(H)), **kw)
    out = np.concatenate([res.results[i]["out"] for i in range(H)], axis=-1)
    return out, res


def kernel(h, A):
    out, _ = run(h, A)
    return out


# revision 4
# speedup vs baseline: 1.0083x; 1.0083x over previous
"""Causal self-attention (per-head A projections) on 8 TRN2 NeuronCores.

Shapes: h [B=2, T=2048, d=64] f32, A [H=8, d, d] f32.
  q = h @ A[i]; scores = q @ h^T (causal); out_i = softmax(scores) @ h.
Sharding: one head per core (embarrassingly parallel, no collectives).
Each core receives the full h and its own A[i]; host concatenates heads.

v3: fp16 score path (1 cyc/col on the PE; f32r measured ~3.3) with bf16
only after the exp (pT/AV need bf16's exponent range).  Masks use -60000
(fp16-representable; exp -> 0).  All score/stats/pre PSUM tiles come from
ONE shared 6-deep ring of [128, 512] banks: measured on HW, a matmul's
completion latency (~450ns) + semaphore hop (~100ns) + consumer (exp or
reduce, ~650ns) means a ring shallower than ~5 stalls the in-order PE
queue at ~2x the stream time.  The 6-deep shared ring keeps the PE at
full rate and lets ring backpressure pace the PE to the DVE/ACT
consumers, which are the true bottleneck.

  PRE(b,c): PE-transpose h chunk (fp16, packed pairs in PSUM cells via a
      bitcast view) into ring tile A; hc copy reads it as 16-bit (DVE 2x).
      qT = A-matmul into ring tile B (f32); ACT copies qc out.  Pool
      casts hs16 (fp16 transpose src); DVE casts hsb (bf16 AV rhs, 4x).
  S(b,g): stats row-max for t-tiles 4g..4g+3.  One ring tile per 512-col
      window; -60000 upper-triangle via identity matmul on the causal
      boundary window; DVE reduce_max per window (negate-folded when the
      tile has a single window); -m (fp16) lands in row 64 of qc via a
      [128,1] SBUF DMA reshape.
  P(b,c): pass2 scoresT tiles [s,t] (fp16, K=65 so the -m row folds the
      subtraction into the matmul), diagonal tiles at exact causal width
      with -60000 from an identity matmul, ACT exp -> pT bf16, AV
      accumulates natural-layout out tiles [t, d+1] in PSUM (ones column
      of hsb = softmax denominator l).  NOTE: matmul start=True clears
      has_written for the WHOLE PSUM bank, so only the first AV matmul
      into each oT bank sets it.  Finalize: one strided DVE reciprocal of
      the l columns, raw oT->SBUF copy (DVE/ACT alternating), per-tile
      scale on GpSimd (keeps the busy engines free), one output DMA.
"""

import os
import sys

for _p in ("/opt/trn_rl_repo",):
    if _p not in sys.path:
        sys.path.insert(0, _p)

import numpy as np
from contextlib import ExitStack

import concourse.bass as bass
import concourse.tile as tile
from concourse import bacc, mybir
from concourse.masks import make_identity
from concourse.bass_utils import run_bass_kernel_spmd

B, T, D, H = 2, 2048, 64, 8
P = 128                # square tile size (t and s)
NT = T // P            # 16 tiles along t/s
CH = 512               # chunk width (PSUM bank)
NCH = T // CH          # 4 chunks
NEG = -60000.0         # fp16-representable -inf stand-in (exp -> 0)

f32 = mybir.dt.float32
fp16 = mybir.dt.float16
bf16 = mybir.dt.bfloat16


def _stat_windows(i):
    """Column windows covering the causal region [0, 128*(i+1)) for t-tile i.

    First window is rem wide (s_end % 512); remaining windows are 512 wide,
    end-aligned so the last one lands exactly on the causal boundary (where
    the mask goes).
    """
    s_end = (i + 1) * P
    rem = s_end % CH
    wins = []
    start = 0
    if rem:
        wins.append((0, rem))
        start = rem
    wins.extend(
        (start + k * CH, start + (k + 1) * CH) for k in range((s_end - start) // CH)
    )
    return wins


def _build(ctx: ExitStack, tc: "tile.TileContext", h_ext, A_ext, out_ext):
    nc = tc.nc

    consts = ctx.enter_context(tc.tile_pool(name="consts", bufs=1))
    hpool = ctx.enter_context(tc.tile_pool(name="hpool", bufs=2))
    qpool = ctx.enter_context(tc.tile_pool(name="qpool", bufs=2))
    ppool = ctx.enter_context(tc.tile_pool(name="ppool", bufs=6))
    spool = ctx.enter_context(tc.tile_pool(name="spool", bufs=4))
    opool = ctx.enter_context(tc.tile_pool(name="opool", bufs=3))
    # PSUM (8 banks): shared 6-deep ring (pre/stats/p2) + oT x2.
    ps_ring = ctx.enter_context(tc.tile_pool(name="ps_ring", bufs=6, space="PSUM"))
    ps_out = ctx.enter_context(tc.tile_pool(name="ps_out", bufs=2, space="PSUM"))

    # ---- constants ----
    identh = consts.tile([P, P], fp16)
    make_identity(nc, identh)

    # umask[t, s] = NEG where s > t else 0 (stats-pass diagonal tile).
    umask = consts.tile([P, P], fp16)
    nc.gpsimd.memset(umask, 0.0)
    nc.gpsimd.affine_select(
        out=umask, in_=umask, compare_op=mybir.AluOpType.is_ge, fill=NEG,
        base=0, channel_multiplier=1, pattern=[[-1, P]],
    )
    # lmask[s, t] = NEG where t < s else 0 (pass-2 diagonal tile, scoresT).
    lmask = consts.tile([P, P], fp16)
    nc.gpsimd.memset(lmask, 0.0)
    nc.gpsimd.affine_select(
        out=lmask, in_=lmask, compare_op=mybir.AluOpType.is_ge, fill=NEG,
        base=0, channel_multiplier=-1, pattern=[[1, P]],
    )
    # A for this core's head: [d, e] natural layout (d on partitions), fp16.
    Asb32 = consts.tile([D, D], f32)
    nc.sync.dma_start(out=Asb32, in_=A_ext)
    Asb = consts.tile([D, D], fp16)
    nc.scalar.copy(Asb, Asb32)

    # ---- bulk input loads (both batches up front) ----
    hs32s, hs16s, hsbs, hcs, qcs = [], [], [], [], []
    for b in range(B):
        hs32 = hpool.tile([P, NT, D + 1], f32, tag="hs32", name=f"hs32_{b}")
        h_re = h_ext[b].rearrange("(j p) d -> p j d", p=P)
        if b == 0:
            # fine-grained first-chunk loads so the first transpose starts
            # as early as possible (shorter pipeline ramp)
            for j in range(4):
                nc.sync.dma_start(out=hs32[:, j, 0:D], in_=h_re[:, j, :])
            nc.sync.dma_start(out=hs32[:, 4:NT, 0:D], in_=h_re[:, 4:NT, :])
        else:
            for c in range(NCH):
                nc.sync.dma_start(
                    out=hs32[:, 4 * c : 4 * c + 4, 0:D],
                    in_=h_re[:, 4 * c : 4 * c + 4, :],
                )
        nc.gpsimd.memset(hs32[:, :, D : D + 1], 1.0)
        hs32s.append(hs32)
        hs16s.append(hpool.tile([P, NT, D + 1], fp16, tag="hs16", name=f"hs16_{b}"))
        hsbs.append(hpool.tile([P, NT, D + 1], bf16, tag="hsb", name=f"hsb_{b}"))
        hcs.append(hpool.tile([D + 1, T], fp16, tag="hc", name=f"hc_{b}"))
        qcs.append(qpool.tile([D + 1, T], fp16, tag="qc", name=f"qc_{b}"))

    def emit_pre(b, c, first=False):
        """Transpose h chunk + qT chunk through two shared-ring PSUM tiles."""
        lo = c * CH
        sl = (slice(None), slice(4 * c, 4 * c + 4), slice(None))
        # fp16 cast for the transpose source: DVE for the ramp-critical first
        # chunk, Pool otherwise; bf16 AV rhs derived on DVE at 4x.
        if first:
            nc.vector.tensor_copy(hs16s[b][sl], hs32s[b][sl])
        else:
            nc.gpsimd.tensor_copy(hs16s[b][sl], hs32s[b][sl])
        nc.vector.tensor_copy(hsbs[b][sl], hs16s[b][sl])

        tA = ps_ring.tile([P, CH], f32, tag="ring", name=f"preT_{b}_{c}")
        pt = tA[0 : D + 1, 0 : CH // 2].bitcast(fp16)
        for r in range(4):
            j = 4 * c + r
            nc.tensor.transpose(pt[:, r * P : (r + 1) * P], hs16s[b][:, j, :], identh)
        hc = hcs[b][:, lo : lo + CH]
        nc.vector.tensor_copy(hc, pt)

        tB = ps_ring.tile([P, CH], f32, tag="ring", name=f"preQ_{b}_{c}")
        pq = tB[0:D, :]
        nc.tensor.matmul(pq, lhsT=Asb, rhs=hc[0:D, :], start=True, stop=True)
        nc.scalar.copy(qcs[b][0:D, lo : lo + CH], pq)

    def emit_stats(b, g):
        """Row-max for t-tiles 4g..4g+3 -> -m into qc row 64."""
        for i in range(4 * g, 4 * g + 4):
            s_end = (i + 1) * P
            wins = _stat_windows(i)
            single = len(wins) == 1
            mxp = spool.tile([P, 4], f32, tag="mxp")
            negm = spool.tile([P, 1], fp16, tag="negm")
            lhs_q = qcs[b][0:D, i * P : (i + 1) * P]
            for wi, (lo, hi) in enumerate(wins):
                w = hi - lo
                flat = ps_ring.tile([P, CH], f32, tag="ring", name=f"st_{b}_{i}_{wi}")
                nc.tensor.matmul(
                    flat[:, 0:w], lhsT=lhs_q, rhs=hcs[b][0:D, lo:hi],
                    start=True, stop=not hi == s_end, skip_group_check=True,
                )
                if hi == s_end:
                    nc.tensor.matmul(
                        flat[:, w - P : w], lhsT=identh, rhs=umask,
                        start=False, stop=True, skip_group_check=True,
                    )
                if single:
                    nc.vector.reduce_max(
                        negm, flat[:, 0:w], axis=mybir.AxisListType.X, negate=True,
                    )
                else:
                    nc.vector.reduce_max(
                        mxp[:, wi : wi + 1], flat[:, 0:w], axis=mybir.AxisListType.X,
                    )
            # Second-level max + negate in one DVE op.
            if not single:
                nc.vector.reduce_max(
                    negm, mxp[:, 0 : len(wins)], axis=mybir.AxisListType.X,
                    negate=True,
                )
            # Partition-column -> free-row reshape via a tiny SBUF->SBUF DMA.
            nc.sync.dma_start(
                out=qcs[b][D : D + 1, i * P : (i + 1) * P], in_=negm
            )

    class P2Chunk:
        """Pass2 scoresT + exp + natural-layout AV + finalize for one chunk.

        Split into per-j steps so two chunks can be emitted zipper-style
        (alternating j-steps) to pipeline the ACT-bound kernel tail.
        """

        def __init__(self, b, c, copy_act=False):
            self.b, self.c = b, c
            self.copy_act = copy_act
            self.oT = ps_out.tile([P, 4, D + 8], f32, tag="oT", name=f"oT_{b}_{c}")
            self.osb = opool.tile([P, 4, D], f32, tag="osb", name=f"osb_{b}_{c}")
            self.av_queue = []
            self.nsteps = 4 * c + 4

        def flush_av(self, limit):
            while len(self.av_queue) > limit:
                jq, pTq = self.av_queue.pop(0)
                for k in range(4):
                    i = 4 * self.c + k
                    if jq > i:
                        continue
                    # start=True clears has_written for the WHOLE bank, so
                    # only the very first matmul into this oT bank may set
                    # it; later first-writes hit cleared bits and overwrite.
                    nc.tensor.matmul(
                        self.oT[:, k, 0 : D + 1], lhsT=pTq[:, k * P : (k + 1) * P],
                        rhs=hsbs[self.b][:, jq, :],
                        start=(jq == 0 and k == 0), stop=(jq == i),
                        skip_group_check=True,
                    )

        def step(self, j):
            b, c = self.b, self.c
            r = j - 4 * c  # >= 0 on diagonal tiles
            diag = r >= 0
            toff = 0 if not diag else P * r  # first causal t column
            p2 = ps_ring.tile([P, CH], f32, tag="ring", name=f"p2_{b}_{c}_{j}")
            nc.tensor.matmul(
                p2[:, toff:CH],
                lhsT=hcs[b][:, j * P : (j + 1) * P],
                rhs=qcs[b][:, c * CH + toff : (c + 1) * CH],
                start=True, stop=not diag, skip_group_check=True,
            )
            if diag:
                # -60000 on the acausal diagonal part, accumulated in PSUM by
                # the PE (keeps Pool out of the exp->AV dependency chain).
                nc.tensor.matmul(
                    p2[:, toff : toff + P], lhsT=identh, rhs=lmask,
                    start=False, stop=True, skip_group_check=True,
                )
            pT = ppool.tile([P, CH], bf16, tag="pT")
            nc.scalar.activation(
                pT[:, toff:CH], p2[:, toff:CH], mybir.ActivationFunctionType.Exp
            )
            self.av_queue.append((j, pT))
            self.flush_av(2)

        def finish(self):
            self.flush_av(0)
            # Finalize: strided reciprocal of the 4 l columns (one DVE op),
            # raw PSUM->SBUF copy, scale on GpSimd, one output DMA.
            oT, osb = self.oT, self.osb
            rl = spool.tile([P, 4], f32, tag="rl")
            nc.vector.reciprocal(rl, oT[:, :, D : D + 1])
            osr = opool.tile([P, 4, D], f32, tag="osr")
            cp = nc.scalar.copy if self.copy_act else nc.vector.tensor_copy
            cp(osr, oT[:, :, 0:D])
            for k in range(4):
                nc.gpsimd.tensor_scalar_mul(
                    osb[:, k, :], osr[:, k, :], rl[:, k : k + 1]
                )
            nc.sync.dma_start(
                out=out_ext[self.b, self.c * CH : (self.c + 1) * CH, :].rearrange(
                    "(j p) d -> p j d", p=P
                ),
                in_=osb,
            )

    def emit_p2(b, c, copy_act=False):
        st = P2Chunk(b, c, copy_act)
        for j in range(st.nsteps):
            st.step(j)
        st.finish()

    def emit_p2_pair(b, c_lo, c_hi):
        """Zipper two chunks' j-loops so exp pipelines through the tail."""
        lo = P2Chunk(b, c_lo)
        hi = P2Chunk(b, c_hi, copy_act=True)
        for j in range(hi.nsteps):
            hi.step(j)
            if j < lo.nsteps:
                lo.step(j)
            if j == lo.nsteps - 1:
                lo.finish()
        hi.finish()

    # ---- unit schedule ----
    # Batch 0 ascends (stats can start after one PRE chunk -> early DVE
    # ramp); batch 1 descends (heaviest stats/exp units mid-kernel, the
    # post-stats tail is only the two smallest chunks, zippered).
    emit_pre(0, 0, first=True)
    emit_stats(0, 0)
    emit_pre(1, 0)
    emit_pre(0, 1)
    emit_stats(0, 1)
    emit_p2(0, 0, copy_act=True)
    emit_pre(0, 2)
    emit_stats(0, 2)
    emit_p2(0, 1, copy_act=True)
    emit_pre(0, 3)
    emit_pre(1, 1)
    emit_stats(0, 3)
    emit_p2(0, 2)
    emit_pre(1, 2)
    emit_pre(1, 3)
    emit_stats(1, 3)
    emit_p2(0, 3)
    emit_p2(1, 3)
    emit_stats(1, 2)
    emit_p2(1, 2)
    emit_stats(1, 1)
    emit_p2(1, 1)
    emit_stats(1, 0)
    emit_p2(1, 0)


_cache = {}


def _get_nc():
    if "nc" not in _cache:
        nc = bacc.Bacc(
            "TRN2", target_bir_lowering=False, debug=False, num_devices=H
        )
        h_ext = nc.dram_tensor("h", [B, T, D], f32, kind="ExternalInput").ap()
        A_ext = nc.dram_tensor("A", [D, D], f32, kind="ExternalInput").ap()
        out_ext = nc.dram_tensor("out", [B, T, D], f32, kind="ExternalOutput").ap()
        with tile.TileContext(nc) as tc:
            with ExitStack() as ctx:
                _build(ctx, tc, h_ext, A_ext, out_ext)
        nc.compile()
        _cache["nc"] = nc
    return _cache["nc"]


def run(h, A, **kw):
    """Run on hardware; returns (full output [B,T,H*D], BassKernelResults)."""
    nc = _get_nc()
    h = np.ascontiguousarray(h, dtype=np.float32)
    A = np.ascontiguousarray(A, dtype=np.float32)
    in_maps = [{"h": h, "A": np.ascontiguousarray(A[i])} for i in range(H)]
    res = run_bass_kernel_spmd(nc, in_maps, core_ids=list(range(H)), **kw)
    out = np.concatenate([res.results[i]["out"] for i in range(H)], axis=-1)
    return out, res


def kernel(h, A):
    out, _ = run(h, A)
    return out


# revision 5
# speedup vs baseline: 1.0686x; 1.0598x over previous
"""Causal self-attention (per-head A projections) on 8 TRN2 NeuronCores.

Shapes: h [B=2, T=2048, d=64] f32, A [H=8, d, d] f32.
  q = h @ A[i]; scores = q @ h^T (causal); out_i = softmax(scores) @ h.
Sharding: one head per core (embarrassingly parallel, no collectives).
Each core receives the full h and its own A[i]; host concatenates heads.

v3: fp16 score path (1 cyc/col on the PE; f32r measured ~3.3) with bf16
only after the exp (pT/AV need bf16's exponent range).  Masks use -60000
(fp16-representable; exp -> 0).  All score/stats/pre PSUM tiles come from
ONE shared 6-deep ring of [128, 512] banks: measured on HW, a matmul's
completion latency (~450ns) + semaphore hop (~100ns) + consumer (exp or
reduce, ~650ns) means a ring shallower than ~5 stalls the in-order PE
queue at ~2x the stream time.  The 6-deep shared ring keeps the PE at
full rate and lets ring backpressure pace the PE to the DVE/ACT
consumers, which are the true bottleneck.

  PRE(b,c): PE-transpose h chunk (fp16, packed pairs in PSUM cells via a
      bitcast view) into ring tile A; hc copy reads it as 16-bit (DVE 2x).
      qT = A-matmul into ring tile B (f32); ACT copies qc out.  Pool
      casts hs16 (fp16 transpose src); DVE casts hsb (bf16 AV rhs, 4x).
  S(b,g): stats row-max for t-tiles 4g..4g+3.  One ring tile per 512-col
      window; -60000 upper-triangle via identity matmul on the causal
      boundary window; DVE reduce_max per window (negate-folded when the
      tile has a single window); -m (fp16) lands in row 64 of qc via a
      [128,1] SBUF DMA reshape.
  P(b,c): pass2 scoresT tiles [s,t] (fp16, K=65 so the -m row folds the
      subtraction into the matmul), diagonal tiles at exact causal width
      with -60000 from an identity matmul, ACT exp -> pT bf16, AV
      accumulates natural-layout out tiles [t, d+1] in PSUM (ones column
      of hsb = softmax denominator l).  NOTE: matmul start=True clears
      has_written for the WHOLE PSUM bank, so only the first AV matmul
      into each oT bank sets it.  Finalize: one strided DVE reciprocal of
      the l columns, raw oT->SBUF copy (DVE/ACT alternating), per-tile
      scale on GpSimd (keeps the busy engines free), one output DMA.
"""

import os
import sys

for _p in ("/opt/trn_rl_repo",):
    if _p not in sys.path:
        sys.path.insert(0, _p)

import numpy as np
from contextlib import ExitStack

import concourse.bass as bass
import concourse.tile as tile
from concourse import bacc, mybir
from concourse.masks import make_identity
from concourse.bass_utils import run_bass_kernel_spmd

B, T, D, H = 2, 2048, 64, 8
P = 128                # square tile size (t and s)
NT = T // P            # 16 tiles along t/s
CH = 512               # chunk width (PSUM bank)
NCH = T // CH          # 4 chunks
NEG = -60000.0         # fp16-representable -inf stand-in (exp -> 0)

f32 = mybir.dt.float32
fp16 = mybir.dt.float16
bf16 = mybir.dt.bfloat16


def _stat_windows(i):
    """Column windows covering the causal region [0, 128*(i+1)) for t-tile i.

    First window is rem wide (s_end % 512); remaining windows are 512 wide,
    end-aligned so the last one lands exactly on the causal boundary (where
    the mask goes).
    """
    s_end = (i + 1) * P
    rem = s_end % CH
    wins = []
    start = 0
    if rem:
        wins.append((0, rem))
        start = rem
    wins.extend(
        (start + k * CH, start + (k + 1) * CH) for k in range((s_end - start) // CH)
    )
    return wins


def _build(ctx: ExitStack, tc: "tile.TileContext", h_ext, A_ext, out_ext):
    nc = tc.nc

    consts = ctx.enter_context(tc.tile_pool(name="consts", bufs=1))
    hpool = ctx.enter_context(tc.tile_pool(name="hpool", bufs=2))
    qpool = ctx.enter_context(tc.tile_pool(name="qpool", bufs=2))
    ppool = ctx.enter_context(tc.tile_pool(name="ppool", bufs=6))
    spool = ctx.enter_context(tc.tile_pool(name="spool", bufs=4))
    opool = ctx.enter_context(tc.tile_pool(name="opool", bufs=3))
    # PSUM (8 banks): shared 6-deep ring (pre/stats/p2) + oT x2.
    ps_ring = ctx.enter_context(tc.tile_pool(name="ps_ring", bufs=6, space="PSUM"))
    ps_out = ctx.enter_context(tc.tile_pool(name="ps_out", bufs=2, space="PSUM"))

    # ---- constants ----
    identh = consts.tile([P, P], fp16)
    make_identity(nc, identh)

    # umask[t, s] = NEG where s > t else 0 (stats-pass diagonal tile).
    umask = consts.tile([P, P], fp16)
    nc.gpsimd.memset(umask, 0.0)
    nc.gpsimd.affine_select(
        out=umask, in_=umask, compare_op=mybir.AluOpType.is_ge, fill=NEG,
        base=0, channel_multiplier=1, pattern=[[-1, P]],
    )
    # lmask[s, t] = NEG where t < s else 0 (pass-2 diagonal tile, scoresT).
    lmask = consts.tile([P, P], fp16)
    nc.gpsimd.memset(lmask, 0.0)
    nc.gpsimd.affine_select(
        out=lmask, in_=lmask, compare_op=mybir.AluOpType.is_ge, fill=NEG,
        base=0, channel_multiplier=-1, pattern=[[1, P]],
    )
    # A for this core's head: [d, e] natural layout (d on partitions), fp16.
    Asb32 = consts.tile([D, D], f32)
    nc.sync.dma_start(out=Asb32, in_=A_ext)
    Asb = consts.tile([D, D], fp16)
    nc.scalar.copy(Asb, Asb32)

    # ---- bulk input loads (both batches up front) ----
    hs32s, hs16s, hsbs, hcs, qcs = [], [], [], [], []
    for b in range(B):
        hs32 = hpool.tile([P, NT, D + 1], f32, tag="hs32", name=f"hs32_{b}")
        h_re = h_ext[b].rearrange("(j p) d -> p j d", p=P)
        if b == 0:
            # fine-grained first-chunk loads so the first transpose starts
            # as early as possible (shorter pipeline ramp)
            for j in range(4):
                nc.sync.dma_start(out=hs32[:, j, 0:D], in_=h_re[:, j, :])
            nc.sync.dma_start(out=hs32[:, 4:NT, 0:D], in_=h_re[:, 4:NT, :])
        else:
            for c in range(NCH):
                nc.sync.dma_start(
                    out=hs32[:, 4 * c : 4 * c + 4, 0:D],
                    in_=h_re[:, 4 * c : 4 * c + 4, :],
                )
        nc.gpsimd.memset(hs32[:, :, D : D + 1], 1.0)
        hs32s.append(hs32)
        hs16s.append(hpool.tile([P, NT, D + 1], fp16, tag="hs16", name=f"hs16_{b}"))
        hsbs.append(hpool.tile([P, NT, D + 1], bf16, tag="hsb", name=f"hsb_{b}"))
        hcs.append(hpool.tile([D + 1, T], fp16, tag="hc", name=f"hc_{b}"))
        qcs.append(qpool.tile([D + 1, T], fp16, tag="qc", name=f"qc_{b}"))

    def emit_pre(b, c, first=False):
        """Transpose h chunk + qT chunk through two shared-ring PSUM tiles."""
        lo = c * CH
        sl = (slice(None), slice(4 * c, 4 * c + 4), slice(None))
        # fp16 cast for the transpose source: DVE for the ramp-critical first
        # chunk, Pool otherwise; bf16 AV rhs derived on DVE at 4x.
        if first:
            nc.vector.tensor_copy(hs16s[b][sl], hs32s[b][sl])
        else:
            nc.gpsimd.tensor_copy(hs16s[b][sl], hs32s[b][sl])
        nc.vector.tensor_copy(hsbs[b][sl], hs16s[b][sl])

        tA = ps_ring.tile([P, CH], f32, tag="ring", name=f"preT_{b}_{c}")
        pt = tA[0 : D + 1, 0 : CH // 2].bitcast(fp16)
        for r in range(4):
            j = 4 * c + r
            nc.tensor.transpose(pt[:, r * P : (r + 1) * P], hs16s[b][:, j, :], identh)
        hc = hcs[b][:, lo : lo + CH]
        nc.vector.tensor_copy(hc, pt)

        tB = ps_ring.tile([P, CH], f32, tag="ring", name=f"preQ_{b}_{c}")
        pq = tB[0:D, :]
        nc.tensor.matmul(pq, lhsT=Asb, rhs=hc[0:D, :], start=True, stop=True)
        nc.scalar.copy(qcs[b][0:D, lo : lo + CH], pq)

    def emit_stats(b, g):
        """Row-max for t-tiles 4g..4g+3 -> -m into qc row 64."""
        for i in range(4 * g, 4 * g + 4):
            s_end = (i + 1) * P
            wins = _stat_windows(i)
            single = len(wins) == 1
            mxp = spool.tile([P, 4], f32, tag="mxp")
            negm = spool.tile([P, 1], fp16, tag="negm")
            lhs_q = qcs[b][0:D, i * P : (i + 1) * P]
            for wi, (lo, hi) in enumerate(wins):
                w = hi - lo
                flat = ps_ring.tile([P, CH], f32, tag="ring", name=f"st_{b}_{i}_{wi}")
                nc.tensor.matmul(
                    flat[:, 0:w], lhsT=lhs_q, rhs=hcs[b][0:D, lo:hi],
                    start=True, stop=not hi == s_end, skip_group_check=True,
                )
                if hi == s_end:
                    nc.tensor.matmul(
                        flat[:, w - P : w], lhsT=identh, rhs=umask,
                        start=False, stop=True, skip_group_check=True,
                    )
                if single:
                    nc.vector.reduce_max(
                        negm, flat[:, 0:w], axis=mybir.AxisListType.X, negate=True,
                    )
                else:
                    nc.vector.reduce_max(
                        mxp[:, wi : wi + 1], flat[:, 0:w], axis=mybir.AxisListType.X,
                    )
            # Second-level max + negate in one DVE op.
            if not single:
                nc.vector.reduce_max(
                    negm, mxp[:, 0 : len(wins)], axis=mybir.AxisListType.X,
                    negate=True,
                )
            # Partition-column -> free-row reshape via a tiny SBUF->SBUF DMA.
            nc.sync.dma_start(
                out=qcs[b][D : D + 1, i * P : (i + 1) * P], in_=negm
            )

    class P2Chunk:
        """Pass2 scoresT + exp + natural-layout AV + finalize for one chunk.

        Split into per-j steps so two chunks can be emitted zipper-style
        (alternating j-steps) to pipeline the ACT-bound kernel tail.
        """

        def __init__(self, b, c, copy_act=False):
            self.b, self.c = b, c
            self.copy_act = copy_act
            self.oT = ps_out.tile([P, 4, D + 8], f32, tag="oT", name=f"oT_{b}_{c}")
            self.osb = opool.tile([P, 4, D], f32, tag="osb", name=f"osb_{b}_{c}")
            self.av_queue = []
            self.nsteps = 4 * c + 4

        def flush_av(self, limit):
            while len(self.av_queue) > limit:
                jq, pTq = self.av_queue.pop(0)
                for k in range(4):
                    i = 4 * self.c + k
                    if jq > i:
                        continue
                    # start=True clears has_written for the WHOLE bank, so
                    # only the very first matmul into this oT bank may set
                    # it; later first-writes hit cleared bits and overwrite.
                    nc.tensor.matmul(
                        self.oT[:, k, 0 : D + 1], lhsT=pTq[:, k * P : (k + 1) * P],
                        rhs=hsbs[self.b][:, jq, :],
                        start=(jq == 0 and k == 0), stop=(jq == i),
                        skip_group_check=True,
                    )

        def step(self, j):
            b, c = self.b, self.c
            r = j - 4 * c  # >= 0 on diagonal tiles
            diag = r >= 0
            toff = 0 if not diag else P * r  # first causal t column
            p2 = ps_ring.tile([P, CH], f32, tag="ring", name=f"p2_{b}_{c}_{j}")
            nc.tensor.matmul(
                p2[:, toff:CH],
                lhsT=hcs[b][:, j * P : (j + 1) * P],
                rhs=qcs[b][:, c * CH + toff : (c + 1) * CH],
                start=True, stop=not diag, skip_group_check=True,
            )
            if diag:
                # -60000 on the acausal diagonal part, accumulated in PSUM by
                # the PE (keeps Pool out of the exp->AV dependency chain).
                nc.tensor.matmul(
                    p2[:, toff : toff + P], lhsT=identh, rhs=lmask,
                    start=False, stop=True, skip_group_check=True,
                )
            pT = ppool.tile([P, CH], bf16, tag="pT")
            nc.scalar.activation(
                pT[:, toff:CH], p2[:, toff:CH], mybir.ActivationFunctionType.Exp
            )
            self.av_queue.append((j, pT))
            self.flush_av(2)

        def finish(self):
            self.flush_av(0)
            # Finalize: strided reciprocal of the 4 l columns (one DVE op),
            # then per-tile scale alternating DVE/ACT, one output DMA.
            oT, osb = self.oT, self.osb
            rl = spool.tile([P, 4], f32, tag="rl")
            nc.vector.reciprocal(rl, oT[:, :, D : D + 1])
            for k in range(4):
                if (k % 2 == 0) != self.copy_act:
                    nc.vector.tensor_scalar_mul(
                        osb[:, k, :], oT[:, k, 0:D], rl[:, k : k + 1]
                    )
                else:
                    nc.scalar.activation(
                        osb[:, k, :], oT[:, k, 0:D],
                        mybir.ActivationFunctionType.Copy, scale=rl[:, k : k + 1],
                    )
            nc.sync.dma_start(
                out=out_ext[self.b, self.c * CH : (self.c + 1) * CH, :].rearrange(
                    "(j p) d -> p j d", p=P
                ),
                in_=osb,
            )

    def emit_p2(b, c, copy_act=False):
        st = P2Chunk(b, c, copy_act)
        for j in range(st.nsteps):
            st.step(j)
        st.finish()

    def emit_p2_pair(b, c_lo, c_hi):
        """Zipper two chunks' j-loops so exp pipelines through the tail."""
        lo = P2Chunk(b, c_lo)
        hi = P2Chunk(b, c_hi, copy_act=True)
        for j in range(hi.nsteps):
            hi.step(j)
            if j < lo.nsteps:
                lo.step(j)
            if j == lo.nsteps - 1:
                lo.finish()
        hi.finish()

    # ---- unit schedule ----
    # Batch 0 ascends (stats can start after one PRE chunk -> early DVE
    # ramp); batch 1 descends (heaviest stats/exp units mid-kernel, the
    # post-stats tail is only the two smallest chunks, zippered).
    emit_pre(0, 0, first=True)
    emit_stats(0, 0)
    emit_pre(1, 0)
    emit_pre(0, 1)
    emit_stats(0, 1)
    emit_p2(0, 0, copy_act=True)
    emit_pre(0, 2)
    emit_stats(0, 2)
    emit_p2(0, 1, copy_act=True)
    emit_pre(0, 3)
    emit_pre(1, 1)
    emit_stats(0, 3)
    emit_p2(0, 2)
    emit_pre(1, 2)
    emit_pre(1, 3)
    emit_stats(1, 3)
    emit_p2(0, 3)
    emit_p2(1, 3)
    emit_stats(1, 2)
    emit_p2(1, 2)
    emit_stats(1, 1)
    emit_p2(1, 1)
    emit_stats(1, 0)
    emit_p2(1, 0)


_cache = {}


def _get_nc():
    if "nc" not in _cache:
        nc = bacc.Bacc(
            "TRN2", target_bir_lowering=False, debug=False, num_devices=H
        )
        h_ext = nc.dram_tensor("h", [B, T, D], f32, kind="ExternalInput").ap()
        A_ext = nc.dram_tensor("A", [D, D], f32, kind="ExternalInput").ap()
        out_ext = nc.dram_tensor("out", [B, T, D], f32, kind="ExternalOutput").ap()
        with tile.TileContext(nc) as tc:
            with ExitStack() as ctx:
                _build(ctx, tc, h_ext, A_ext, out_ext)
        nc.compile()
        _cache["nc"] = nc
    return _cache["nc"]


def run(h, A, **kw):
    """Run on hardware; returns (full output [B,T,H*D], BassKernelResults)."""
    nc = _get_nc()
    h = np.ascontiguousarray(h, dtype=np.float32)
    A = np.ascontiguousarray(A, dtype=np.float32)
    in_maps = [{"h": h, "A": np.ascontiguousarray(A[i])} for i in range(H)]
    res = run_bass_kernel_spmd(nc, in_maps, core_ids=list(range(H)), **kw)
    out = np.concatenate([res.results[i]["out"] for i in range(H)], axis=-1)
    return out, res


def kernel(h, A):
    out, _ = run(h, A)
    return out


# revision 7
# speedup vs baseline: 1.1726x; 1.0973x over previous
"""Causal self-attention (per-head A projections) on 8 TRN2 NeuronCores.

Shapes: h [B=2, T=2048, d=64] f32, A [H=8, d, d] f32.
  q = h @ A[i]; scores = q @ h^T (causal); out_i = softmax(scores) @ h.
Sharding: one head per core (embarrassingly parallel, no collectives).
Each core receives the full h and its own A[i]; host concatenates heads.

v4: fp16 score path (1 cyc/col on the PE; f32r measured ~3.3 cyc/col on
HW) with bf16 only after the exp (pT/AV need bf16's exponent range);
masks use -60000 (fp16-representable; exp -> 0).

The kernel is throughput-bound on the exp (ACT) and row-max reduce (DVE)
consumers, not the PE.  Two structural rules follow:

1. Wide consumer ops: stats window PAIRS and p2 step PAIRS share one
   2-bank PSUM tile, so ONE DVE reduce / ONE ACT exp covers 1024
   columns (the ~200-cycle per-op PSUM overhead halves).

2. Fine-grained interleave: each p2 pair is emitted back-to-back with a
   stats pair from a LATER unit (Bresenham-merged), so the ACT exp
   stream and the DVE reduce stream run concurrently instead of
   alternating unit-by-unit through the in-order PE queue.  Schedule:
   batch 0 chunks ascend while their successor stats interleave; batch 1
   descends (its stats run against batch 0's largest p2 chunks).

PSUM (8 banks): shared 3-deep ring of [128, 1024] 2-bank tiles
(pre/stats/p2 - a matmul's ~450ns completion latency + ~100ns semaphore
+ ~1.1us consumer would stall a shallower dedicated ring) + oT x2.

  PRE(b,c): PE-transpose h chunk (fp16, packed pairs in PSUM cells via a
      bitcast view) into bank 0 of a ring tile; DVE copies hc out as
      16-bit; qT = A-matmul into bank 1 (f32); ACT copies qc out.  Pool
      casts hs16 (fp16 transpose src); DVE casts hsb (bf16 AV rhs, 4x).
  S(b,g): stats row-max for t-tiles 4g..4g+3, window pairs packed
      against the bank boundary so one DVE reduce consumes each pair;
      -60000 upper-triangle via identity matmul on the causal boundary
      window; -m (fp16) lands in row 64 of qc via a [128,1] SBUF DMA.
  P(b,c): pass2 scoresT [s,t] step pairs (fp16, K=65 so the -m row folds
      the shift into the matmul), diagonal tiles at exact causal width
      with -60000 from an identity matmul; ONE ACT exp per pair -> pT
      bf16 (the second half's acausal head may exp garbage; those pT
      columns are never read by AV); AV accumulates natural-layout out
      tiles [t, d+1] in PSUM (ones column of hsb = denominator l).
      NOTE: matmul start=True clears has_written for the WHOLE PSUM
      bank, so only the first AV matmul into each oT bank sets it.
      Finalize: one strided DVE reciprocal of the 4 l columns + per-tile
      scale (DVE/ACT alternating), one output DMA per chunk.
"""

import os
import sys

for _p in ("/opt/trn_rl_repo",):
    if _p not in sys.path:
        sys.path.insert(0, _p)

import numpy as np
from contextlib import ExitStack

import concourse.bass as bass
import concourse.tile as tile
from concourse import bacc, mybir
from concourse.masks import make_identity
from concourse.bass_utils import run_bass_kernel_spmd

B, T, D, H = 2, 2048, 64, 8
P = 128                # square tile size (t and s)
NT = T // P            # 16 tiles along t/s
CH = 512               # chunk width (PSUM bank)
NCH = T // CH          # 4 chunks
NEG = -60000.0         # fp16-representable -inf stand-in (exp -> 0)

f32 = mybir.dt.float32
fp16 = mybir.dt.float16
bf16 = mybir.dt.bfloat16


def _stat_windows(i):
    """Column windows covering the causal region [0, 128*(i+1)) for t-tile i.

    First window is rem wide (s_end % 512); remaining windows are 512 wide,
    end-aligned so the last one lands exactly on the causal boundary (where
    the mask goes).
    """
    s_end = (i + 1) * P
    rem = s_end % CH
    wins = []
    start = 0
    if rem:
        wins.append((0, rem))
        start = rem
    wins.extend(
        (start + k * CH, start + (k + 1) * CH) for k in range((s_end - start) // CH)
    )
    return wins


def _bresenham_merge(a, b):
    """Proportionally interleave two emitter lists (a paced against b)."""
    out = []
    na, nb = len(a), len(b)
    if not na:
        return list(b)
    if not nb:
        return list(a)
    ia = ib = 0
    err = 0
    while ia < na or ib < nb:
        if ib >= nb or (ia < na and err * nb < na * (2 * ib + 1) - nb):
            out.append(a[ia]); ia += 1; err += 2 * nb
        else:
            out.append(b[ib]); ib += 1
    return out


def _build(ctx: ExitStack, tc: "tile.TileContext", h_ext, A_ext, out_ext):
    nc = tc.nc

    consts = ctx.enter_context(tc.tile_pool(name="consts", bufs=1))
    hpool = ctx.enter_context(tc.tile_pool(name="hpool", bufs=2))
    qpool = ctx.enter_context(tc.tile_pool(name="qpool", bufs=2))
    ppool = ctx.enter_context(tc.tile_pool(name="ppool", bufs=4))
    spool = ctx.enter_context(tc.tile_pool(name="spool", bufs=4))
    opool = ctx.enter_context(tc.tile_pool(name="opool", bufs=2))
    # PSUM (8 banks): shared 3-deep ring of 2-bank tiles + oT x2.
    ps_ring = ctx.enter_context(tc.tile_pool(name="ps_ring", bufs=3, space="PSUM"))
    ps_out = ctx.enter_context(tc.tile_pool(name="ps_out", bufs=2, space="PSUM"))

    # ---- constants ----
    identh = consts.tile([P, P], fp16)
    make_identity(nc, identh)

    # umask[t, s] = NEG where s > t else 0 (stats-pass diagonal tile).
    umask = consts.tile([P, P], fp16)
    nc.gpsimd.memset(umask, 0.0)
    nc.gpsimd.affine_select(
        out=umask, in_=umask, compare_op=mybir.AluOpType.is_ge, fill=NEG,
        base=0, channel_multiplier=1, pattern=[[-1, P]],
    )
    # lmask[s, t] = NEG where t < s else 0 (pass-2 diagonal tile, scoresT).
    lmask = consts.tile([P, P], fp16)
    nc.gpsimd.memset(lmask, 0.0)
    nc.gpsimd.affine_select(
        out=lmask, in_=lmask, compare_op=mybir.AluOpType.is_ge, fill=NEG,
        base=0, channel_multiplier=-1, pattern=[[1, P]],
    )
    # A for this core's head: [d, e] natural layout (d on partitions), fp16.
    Asb32 = consts.tile([D, D], f32)
    nc.sync.dma_start(out=Asb32, in_=A_ext)
    Asb = consts.tile([D, D], fp16)
    nc.scalar.copy(Asb, Asb32)

    # ---- bulk input loads (both batches up front) ----
    hs32s, hs16s, hsbs, hcs, qcs = [], [], [], [], []
    for b in range(B):
        hs32 = hpool.tile([P, NT, D + 1], f32, tag="hs32", name=f"hs32_{b}")
        h_re = h_ext[b].rearrange("(j p) d -> p j d", p=P)
        if b == 0:
            # fine-grained first-chunk loads so the first transpose starts
            # as early as possible (shorter pipeline ramp)
            for j in range(4):
                nc.sync.dma_start(out=hs32[:, j, 0:D], in_=h_re[:, j, :])
            nc.sync.dma_start(out=hs32[:, 4:NT, 0:D], in_=h_re[:, 4:NT, :])
        else:
            for c in range(NCH):
                nc.sync.dma_start(
                    out=hs32[:, 4 * c : 4 * c + 4, 0:D],
                    in_=h_re[:, 4 * c : 4 * c + 4, :],
                )
        nc.gpsimd.memset(hs32[:, :, D : D + 1], 1.0)
        hs32s.append(hs32)
        hs16s.append(hpool.tile([P, NT, D + 1], fp16, tag="hs16", name=f"hs16_{b}"))
        hsbs.append(hpool.tile([P, NT, D + 1], bf16, tag="hsb", name=f"hsb_{b}"))
        hcs.append(hpool.tile([D + 1, T], fp16, tag="hc", name=f"hc_{b}"))
        qcs.append(qpool.tile([D + 1, T], fp16, tag="qc", name=f"qc_{b}"))

    def emit_pre(b, c, first=False):
        """Transpose h chunk + qT chunk through one 2-bank ring PSUM tile."""
        lo = c * CH
        sl = (slice(None), slice(4 * c, 4 * c + 4), slice(None))
        # fp16 cast for the transpose source: DVE for the ramp-critical first
        # chunk, Pool otherwise; bf16 AV rhs derived on DVE at 4x.
        if first:
            nc.vector.tensor_copy(hs16s[b][sl], hs32s[b][sl])
        else:
            nc.gpsimd.tensor_copy(hs16s[b][sl], hs32s[b][sl])
        nc.vector.tensor_copy(hsbs[b][sl], hs16s[b][sl])

        stile = ps_ring.tile([P, 2 * CH], f32, tag="ring", name=f"pre_{b}_{c}")
        pt = stile[0 : D + 1, 0 : CH // 2].bitcast(fp16)
        for r in range(4):
            j = 4 * c + r
            nc.tensor.transpose(pt[:, r * P : (r + 1) * P], hs16s[b][:, j, :], identh)
        hc = hcs[b][:, lo : lo + CH]
        nc.vector.tensor_copy(hc, pt)

        pq = stile[0:D, CH : 2 * CH]
        nc.tensor.matmul(pq, lhsT=Asb, rhs=hc[0:D, :], start=True, stop=True)
        nc.scalar.copy(qcs[b][0:D, lo : lo + CH], pq)

    def stats_emitters(b, g):
        """Per-PAIR emitters for the row-max of t-tiles 4g..4g+3."""
        ems = []
        for i in range(4 * g, 4 * g + 4):
            wins = _stat_windows(i)
            npair = (len(wins) + 1) // 2
            state = {}

            def make(i, p0, wins, npair, state):
                def em():
                    s_end = (i + 1) * P
                    if p0 == 0:
                        state["mxp"] = spool.tile([P, 2], f32, tag="mxp", name=f"mxp_{b}_{i}")
                        state["negm"] = spool.tile([P, 1], fp16, tag="negm", name=f"negm_{b}_{i}")
                        state["nred"] = 0
                    pair = wins[p0 : p0 + 2]
                    flat = ps_ring.tile([P, 2 * CH], f32, tag="ring",
                                        name=f"st_{b}_{i}_{p0}")
                    if len(pair) == 2:
                        (lo0, hi0), (lo1, hi1) = pair
                        w0, w1 = hi0 - lo0, hi1 - lo1
                        spans = [(lo0, hi0, CH - w0, CH), (lo1, hi1, CH, CH + w1)]
                    else:
                        (lo0, hi0) = pair[0]
                        spans = [(lo0, hi0, 0, hi0 - lo0)]
                    lhs_q = qcs[b][0:D, i * P : (i + 1) * P]
                    for lo, hi, a, bnd in spans:
                        nc.tensor.matmul(
                            flat[:, a:bnd], lhsT=lhs_q, rhs=hcs[b][0:D, lo:hi],
                            start=True, stop=not hi == s_end, skip_group_check=True,
                        )
                        if hi == s_end:
                            nc.tensor.matmul(
                                flat[:, bnd - P : bnd], lhsT=identh, rhs=umask,
                                start=False, stop=True, skip_group_check=True,
                            )
                    a0, a1 = spans[0][2], spans[-1][3]
                    if npair == 1:
                        nc.vector.reduce_max(
                            state["negm"], flat[:, a0:a1],
                            axis=mybir.AxisListType.X, negate=True,
                        )
                    else:
                        nc.vector.reduce_max(
                            state["mxp"][:, state["nred"] : state["nred"] + 1],
                            flat[:, a0:a1], axis=mybir.AxisListType.X,
                        )
                    state["nred"] += 1
                    if p0 + 2 >= len(wins):
                        if npair > 1:
                            nc.vector.reduce_max(
                                state["negm"], state["mxp"][:, 0 : state["nred"]],
                                axis=mybir.AxisListType.X, negate=True,
                            )
                        # Partition-column -> free-row reshape via tiny DMA.
                        nc.sync.dma_start(
                            out=qcs[b][D : D + 1, i * P : (i + 1) * P],
                            in_=state["negm"],
                        )
                return em

            for p0 in range(0, len(wins), 2):
                ems.append(make(i, p0, wins, npair, state))
        return ems

    class P2Chunk:
        """Pass2 scoresT + exp + natural-layout AV + finalize for one chunk."""

        def __init__(self, b, c, scale_act=False):
            self.b, self.c = b, c
            self.scale_act = scale_act
            self.oT = ps_out.tile([P, 4, D + 8], f32, tag="oT", name=f"oT_{b}_{c}")
            self.osb = opool.tile([P, 4, D], f32, tag="osb", name=f"osb_{b}_{c}")
            self.av_queue = []
            self.nsteps = 4 * c + 4

        def flush_av(self, limit):
            while len(self.av_queue) > limit:
                jq, pTh = self.av_queue.pop(0)
                for k in range(4):
                    i = 4 * self.c + k
                    if jq > i:
                        continue
                    # start=True clears has_written for the WHOLE bank, so
                    # only the very first matmul into this oT bank may set
                    # it; later first-writes hit cleared bits and overwrite.
                    nc.tensor.matmul(
                        self.oT[:, k, 0 : D + 1], lhsT=pTh[:, k * P : (k + 1) * P],
                        rhs=hsbs[self.b][:, jq, :],
                        start=(jq == 0 and k == 0), stop=(jq == i),
                        skip_group_check=True,
                    )

        def pair(self, j):
            """Steps j, j+1 into one 2-bank tile; ONE exp covers both."""
            b, c = self.b, self.c
            p2 = ps_ring.tile([P, 2 * CH], f32, tag="ring", name=f"p2_{b}_{c}_{j}")
            toffs = []
            for half, jj in enumerate((j, j + 1)):
                r = jj - 4 * c
                diag = r >= 0
                toff = 0 if not diag else P * r
                base = half * CH
                nc.tensor.matmul(
                    p2[:, base + toff : base + CH],
                    lhsT=hcs[b][:, jj * P : (jj + 1) * P],
                    rhs=qcs[b][:, c * CH + toff : (c + 1) * CH],
                    start=True, stop=not diag, skip_group_check=True,
                )
                if diag:
                    nc.tensor.matmul(
                        p2[:, base + toff : base + toff + P], lhsT=identh,
                        rhs=lmask, start=False, stop=True, skip_group_check=True,
                    )
                toffs.append(toff)
            pT = ppool.tile([P, 2 * CH], bf16, tag="pT")
            # One exp over both halves; the second half's [CH : CH+toff1)
            # may exponentiate stale PSUM, but AV never reads those columns.
            nc.scalar.activation(
                pT[:, toffs[0] : 2 * CH], p2[:, toffs[0] : 2 * CH],
                mybir.ActivationFunctionType.Exp,
            )
            self.av_queue.append((j, pT[:, 0:CH]))
            self.av_queue.append((j + 1, pT[:, CH : 2 * CH]))
            self.flush_av(2)

        def emitters(self):
            ems = [lambda jj=j: self.pair(jj) for j in range(0, self.nsteps, 2)]
            ems.append(self.finish)
            return ems

        def finish(self):
            self.flush_av(0)
            # Finalize: strided reciprocal of the 4 l columns (one DVE op),
            # then per-tile scale alternating DVE/ACT, one output DMA.
            oT, osb = self.oT, self.osb
            rl = spool.tile([P, 4], f32, tag="rl")
            nc.vector.reciprocal(rl, oT[:, :, D : D + 1])
            for k in range(4):
                if (k % 2 == 0) != self.scale_act:
                    nc.vector.tensor_scalar_mul(
                        osb[:, k, :], oT[:, k, 0:D], rl[:, k : k + 1]
                    )
                else:
                    nc.scalar.activation(
                        osb[:, k, :], oT[:, k, 0:D],
                        mybir.ActivationFunctionType.Copy, scale=rl[:, k : k + 1],
                    )
            nc.sync.dma_start(
                out=out_ext[self.b, self.c * CH : (self.c + 1) * CH, :].rearrange(
                    "(j p) d -> p j d", p=P
                ),
                in_=osb,
            )

    # ---- interleaved schedule ----
    # Ramp: batch-0 chunk 0/1 pre + stats(0,0); then zip rounds pairing
    # each p2 chunk with the NEXT stats group so the ACT exp stream and
    # DVE reduce stream run concurrently; pre blobs ride along.
    emit_pre(0, 0, first=True)
    for em in stats_emitters(0, 0):
        em()
    emit_pre(0, 1)

    def p2_ems(b, c, scale_act=False):
        return P2Chunk(b, c, scale_act).emitters()

    rounds = [
        (p2_ems(0, 0, scale_act=True), stats_emitters(0, 1), [lambda: emit_pre(1, 0)]),
        (p2_ems(0, 1, scale_act=True), stats_emitters(0, 2), [lambda: emit_pre(0, 2)]),
        (p2_ems(0, 2), stats_emitters(0, 3), [lambda: emit_pre(0, 3),
                                              lambda: emit_pre(1, 1)]),
        (p2_ems(0, 3), stats_emitters(1, 3), [lambda: emit_pre(1, 2),
                                              lambda: emit_pre(1, 3)]),
        (p2_ems(1, 3), stats_emitters(1, 2), []),
        (p2_ems(1, 2), stats_emitters(1, 1), []),
        (p2_ems(1, 1), stats_emitters(1, 0), []),
        (p2_ems(1, 0), [], []),
    ]
    for p2l, stl, prel in rounds:
        # pre blobs lead (their hc/qc feed the interleaved stats windows of
        # LATER rounds), then p2 pairs proportionally merged with stats.
        for em in prel:
            em()
        for em in _bresenham_merge(p2l, stl):
            em()


_cache = {}


def _get_nc():
    if "nc" not in _cache:
        nc = bacc.Bacc(
            "TRN2", target_bir_lowering=False, debug=False, num_devices=H
        )
        h_ext = nc.dram_tensor("h", [B, T, D], f32, kind="ExternalInput").ap()
        A_ext = nc.dram_tensor("A", [D, D], f32, kind="ExternalInput").ap()
        out_ext = nc.dram_tensor("out", [B, T, D], f32, kind="ExternalOutput").ap()
        with tile.TileContext(nc) as tc:
            with ExitStack() as ctx:
                _build(ctx, tc, h_ext, A_ext, out_ext)
        nc.compile()
        _cache["nc"] = nc
    return _cache["nc"]


def run(h, A, **kw):
    """Run on hardware; returns (full output [B,T,H*D], BassKernelResults)."""
    nc = _get_nc()
    h = np.ascontiguousarray(h, dtype=np.float32)
    A = np.ascontiguousarray(A, dtype=np.float32)
    in_maps = [{"h": h, "A": np.ascontiguousarray(A[i])} for i in range(H)]
    res = run_bass_kernel_spmd(nc, in_maps, core_ids=list(range(H)), **kw)
    out = np.concatenate([res.results[i]["out"] for i in range(H)], axis=-1)
    return out, res


def kernel(h, A):
    out, _ = run(h, A)
    return out


# revision 8
# speedup vs baseline: 1.2472x; 1.0636x over previous
"""Causal self-attention (per-head A projections) on 8 TRN2 NeuronCores.

Shapes: h [B=2, T=2048, d=64] f32, A [H=8, d, d] f32.
  q = h @ A[i]; scores = q @ h^T (causal); out_i = softmax(scores) @ h.
Sharding: one head per core (embarrassingly parallel, no collectives).
Each core receives the full h and its own A[i]; host concatenates heads.

v4: fp16 score path (1 cyc/col on the PE; f32r measured ~3.3 cyc/col on
HW) with bf16 only after the exp (pT/AV need bf16's exponent range);
masks use -60000 (fp16-representable; exp -> 0).

The kernel is throughput-bound on the exp (ACT) and row-max reduce (DVE)
consumers, not the PE.  Two structural rules follow:

1. Wide consumer ops: stats window PAIRS and p2 step PAIRS share one
   2-bank PSUM tile, so ONE DVE reduce / ONE ACT exp covers 1024
   columns (the ~200-cycle per-op PSUM overhead halves).

2. Fine-grained interleave: each p2 pair is emitted back-to-back with a
   stats pair from a LATER unit (Bresenham-merged), so the ACT exp
   stream and the DVE reduce stream run concurrently instead of
   alternating unit-by-unit through the in-order PE queue.  Schedule:
   batch 0 chunks ascend while their successor stats interleave; batch 1
   descends (its stats run against batch 0's largest p2 chunks).

PSUM (8 banks): shared 3-deep ring of [128, 1024] 2-bank tiles
(pre/stats/p2 - a matmul's ~450ns completion latency + ~100ns semaphore
+ ~1.1us consumer would stall a shallower dedicated ring) + oT x2.

  PRE(b,c): PE-transpose h chunk (fp16, packed pairs in PSUM cells via a
      bitcast view) into bank 0 of a ring tile; DVE copies hc out as
      16-bit; qT = A-matmul into bank 1 (f32); ACT copies qc out.  Pool
      casts hs16 (fp16 transpose src); DVE casts hsb (bf16 AV rhs, 4x).
  S(b,g): stats row-max for t-tiles 4g..4g+3, window pairs packed
      against the bank boundary so one DVE reduce consumes each pair;
      -60000 upper-triangle via identity matmul on the causal boundary
      window; -m (fp16) lands in row 64 of qc via a [128,1] SBUF DMA.
  P(b,c): pass2 scoresT [s,t] step pairs (fp16, K=65 so the -m row folds
      the shift into the matmul), diagonal tiles at exact causal width
      with -60000 from an identity matmul; ONE ACT exp per pair -> pT
      bf16 (the second half's acausal head may exp garbage; those pT
      columns are never read by AV); AV accumulates natural-layout out
      tiles [t, d+1] in PSUM (ones column of hsb = denominator l).
      NOTE: matmul start=True clears has_written for the WHOLE PSUM
      bank, so only the first AV matmul into each oT bank sets it.
      Finalize: one strided DVE reciprocal of the 4 l columns + per-tile
      scale (DVE/ACT alternating), one output DMA per chunk.
"""

import os
import sys

for _p in ("/opt/trn_rl_repo",):
    if _p not in sys.path:
        sys.path.insert(0, _p)

import numpy as np
from contextlib import ExitStack

import concourse.bass as bass
import concourse.tile as tile
from concourse import bacc, mybir
from concourse.masks import make_identity
from concourse.bass_utils import run_bass_kernel_spmd

B, T, D, H = 2, 2048, 64, 8
P = 128                # square tile size (t and s)
NT = T // P            # 16 tiles along t/s
CH = 512               # chunk width (PSUM bank)
NCH = T // CH          # 4 chunks
NEG = -60000.0         # fp16-representable -inf stand-in (exp -> 0)

f32 = mybir.dt.float32
fp16 = mybir.dt.float16
bf16 = mybir.dt.bfloat16


def _stat_windows(i):
    """Column windows covering the causal region [0, 128*(i+1)) for t-tile i.

    First window is rem wide (s_end % 512); remaining windows are 512 wide,
    end-aligned so the last one lands exactly on the causal boundary (where
    the mask goes).
    """
    s_end = (i + 1) * P
    rem = s_end % CH
    wins = []
    start = 0
    if rem:
        wins.append((0, rem))
        start = rem
    wins.extend(
        (start + k * CH, start + (k + 1) * CH) for k in range((s_end - start) // CH)
    )
    return wins


def _bresenham_merge(a, b):
    """Proportionally interleave two emitter lists (a paced against b)."""
    out = []
    na, nb = len(a), len(b)
    if not na:
        return list(b)
    if not nb:
        return list(a)
    ia = ib = 0
    err = 0
    while ia < na or ib < nb:
        if ib >= nb or (ia < na and err * nb < na * (2 * ib + 1) - nb):
            out.append(a[ia]); ia += 1; err += 2 * nb
        else:
            out.append(b[ib]); ib += 1
    return out


def _build(ctx: ExitStack, tc: "tile.TileContext", h_ext, A_ext, out_ext):
    nc = tc.nc

    consts = ctx.enter_context(tc.tile_pool(name="consts", bufs=1))
    hpool = ctx.enter_context(tc.tile_pool(name="hpool", bufs=2))
    qpool = ctx.enter_context(tc.tile_pool(name="qpool", bufs=2))
    ppool = ctx.enter_context(tc.tile_pool(name="ppool", bufs=4))
    spool = ctx.enter_context(tc.tile_pool(name="spool", bufs=4))
    opool = ctx.enter_context(tc.tile_pool(name="opool", bufs=2))
    # PSUM (8 banks): shared 3-deep ring of 2-bank tiles + oT x2.
    ps_ring = ctx.enter_context(tc.tile_pool(name="ps_ring", bufs=3, space="PSUM"))
    ps_out = ctx.enter_context(tc.tile_pool(name="ps_out", bufs=2, space="PSUM"))

    # ---- constants ----
    identh = consts.tile([P, P], fp16)
    make_identity(nc, identh)

    # umask[t, s] = NEG where s > t else 0 (stats-pass diagonal tile).
    umask = consts.tile([P, P], fp16)
    nc.gpsimd.memset(umask, 0.0)
    nc.gpsimd.affine_select(
        out=umask, in_=umask, compare_op=mybir.AluOpType.is_ge, fill=NEG,
        base=0, channel_multiplier=1, pattern=[[-1, P]],
    )
    # lmask[s, t] = NEG where t < s else 0 (pass-2 diagonal tile, scoresT).
    lmask = consts.tile([P, P], fp16)
    nc.gpsimd.memset(lmask, 0.0)
    nc.gpsimd.affine_select(
        out=lmask, in_=lmask, compare_op=mybir.AluOpType.is_ge, fill=NEG,
        base=0, channel_multiplier=-1, pattern=[[1, P]],
    )
    # A for this core's head: [d, e] natural layout (d on partitions), fp16.
    Asb32 = consts.tile([D, D], f32)
    nc.sync.dma_start(out=Asb32, in_=A_ext)
    Asb = consts.tile([D, D], fp16)
    nc.scalar.copy(Asb, Asb32)

    # HAM warmup: ~64 constant K=128 matmuls (~5-7us) while the input DMAs
    # land.  The PE clock gate defaults to 4/8 (1.2 GHz) and only a
    # sustained ~3.4us busy window at high array activity releases it;
    # without this the whole kernel streams matmuls at half clock.
    wtile = ps_ring.tile([P, 2 * CH], f32, tag="ring", name="warm")
    for r in range(64):
        nc.tensor.matmul(
            wtile[:, 0:P], lhsT=identh, rhs=umask,
            start=True, stop=True, skip_group_check=True,
        )

    # ---- bulk input loads (both batches up front) ----
    hs32s, hs16s, hsbs, hcs, qcs = [], [], [], [], []
    for b in range(B):
        hs32 = hpool.tile([P, NT, D + 1], f32, tag="hs32", name=f"hs32_{b}")
        h_re = h_ext[b].rearrange("(j p) d -> p j d", p=P)
        if b == 0:
            # fine-grained first-chunk loads so the first transpose starts
            # as early as possible (shorter pipeline ramp)
            for j in range(4):
                nc.sync.dma_start(out=hs32[:, j, 0:D], in_=h_re[:, j, :])
            nc.sync.dma_start(out=hs32[:, 4:NT, 0:D], in_=h_re[:, 4:NT, :])
        else:
            for c in range(NCH):
                nc.sync.dma_start(
                    out=hs32[:, 4 * c : 4 * c + 4, 0:D],
                    in_=h_re[:, 4 * c : 4 * c + 4, :],
                )
        nc.gpsimd.memset(hs32[:, :, D : D + 1], 1.0)
        hs32s.append(hs32)
        hs16s.append(hpool.tile([P, NT, D + 1], fp16, tag="hs16", name=f"hs16_{b}"))
        hsbs.append(hpool.tile([P, NT, D + 1], bf16, tag="hsb", name=f"hsb_{b}"))
        hcs.append(hpool.tile([D + 1, T], fp16, tag="hc", name=f"hc_{b}"))
        qcs.append(qpool.tile([D + 1, T], fp16, tag="qc", name=f"qc_{b}"))

    def emit_pre(b, c, first=False):
        """Transpose h chunk + qT chunk through one 2-bank ring PSUM tile."""
        lo = c * CH
        sl = (slice(None), slice(4 * c, 4 * c + 4), slice(None))
        # fp16 cast for the transpose source: DVE for the ramp-critical first
        # chunk, Pool otherwise; bf16 AV rhs derived on DVE at 4x.
        if first:
            nc.vector.tensor_copy(hs16s[b][sl], hs32s[b][sl])
            nc.vector.tensor_copy(hsbs[b][sl], hs16s[b][sl])
        else:
            nc.gpsimd.tensor_copy(hs16s[b][sl], hs32s[b][sl])
            nc.gpsimd.tensor_copy(hsbs[b][sl], hs16s[b][sl])

        stile = ps_ring.tile([P, 2 * CH], f32, tag="ring", name=f"pre_{b}_{c}")
        pt = stile[0 : D + 1, 0 : CH // 2].bitcast(fp16)
        for r in range(4):
            j = 4 * c + r
            nc.tensor.transpose(pt[:, r * P : (r + 1) * P], hs16s[b][:, j, :], identh)
        hc = hcs[b][:, lo : lo + CH]
        nc.vector.tensor_copy(hc, pt)

        pq = stile[0:D, CH : 2 * CH]
        nc.tensor.matmul(pq, lhsT=Asb, rhs=hc[0:D, :], start=True, stop=True)
        nc.scalar.copy(qcs[b][0:D, lo : lo + CH], pq)

    def stats_emitters(b, g):
        """Per-PAIR emitters for the row-max of t-tiles 4g..4g+3."""
        ems = []
        for i in range(4 * g, 4 * g + 4):
            wins = _stat_windows(i)
            npair = (len(wins) + 1) // 2
            state = {}

            def make(i, p0, wins, npair, state):
                def em():
                    s_end = (i + 1) * P
                    if p0 == 0:
                        state["mxp"] = spool.tile([P, 2], f32, tag="mxp", name=f"mxp_{b}_{i}")
                        state["negm"] = spool.tile([P, 1], fp16, tag="negm", name=f"negm_{b}_{i}")
                        state["nred"] = 0
                    pair = wins[p0 : p0 + 2]
                    flat = ps_ring.tile([P, 2 * CH], f32, tag="ring",
                                        name=f"st_{b}_{i}_{p0}")
                    if len(pair) == 2:
                        (lo0, hi0), (lo1, hi1) = pair
                        w0, w1 = hi0 - lo0, hi1 - lo1
                        spans = [(lo0, hi0, CH - w0, CH), (lo1, hi1, CH, CH + w1)]
                    else:
                        (lo0, hi0) = pair[0]
                        spans = [(lo0, hi0, 0, hi0 - lo0)]
                    lhs_q = qcs[b][0:D, i * P : (i + 1) * P]
                    for lo, hi, a, bnd in spans:
                        nc.tensor.matmul(
                            flat[:, a:bnd], lhsT=lhs_q, rhs=hcs[b][0:D, lo:hi],
                            start=True, stop=not hi == s_end, skip_group_check=True,
                        )
                        if hi == s_end:
                            nc.tensor.matmul(
                                flat[:, bnd - P : bnd], lhsT=identh, rhs=umask,
                                start=False, stop=True, skip_group_check=True,
                            )
                    a0, a1 = spans[0][2], spans[-1][3]
                    if npair == 1:
                        nc.vector.reduce_max(
                            state["negm"], flat[:, a0:a1],
                            axis=mybir.AxisListType.X, negate=True,
                        )
                    else:
                        nc.vector.reduce_max(
                            state["mxp"][:, state["nred"] : state["nred"] + 1],
                            flat[:, a0:a1], axis=mybir.AxisListType.X,
                        )
                    state["nred"] += 1
                    if p0 + 2 >= len(wins):
                        if npair > 1:
                            nc.vector.reduce_max(
                                state["negm"], state["mxp"][:, 0 : state["nred"]],
                                axis=mybir.AxisListType.X, negate=True,
                            )
                        # Partition-column -> free-row reshape via tiny DMA.
                        nc.sync.dma_start(
                            out=qcs[b][D : D + 1, i * P : (i + 1) * P],
                            in_=state["negm"],
                        )
                return em

            for p0 in range(0, len(wins), 2):
                ems.append(make(i, p0, wins, npair, state))
        return ems

    class P2Chunk:
        """Pass2 scoresT + exp + natural-layout AV + finalize for one chunk."""

        def __init__(self, b, c, scale_act=False):
            self.b, self.c = b, c
            self.scale_act = scale_act
            self.oT = ps_out.tile([P, 4, D + 8], f32, tag="oT", name=f"oT_{b}_{c}")
            self.osb = opool.tile([P, 4, D], f32, tag="osb", name=f"osb_{b}_{c}")
            self.av_queue = []
            self.nsteps = 4 * c + 4

        def flush_av(self, limit):
            while len(self.av_queue) > limit:
                jq, pTh = self.av_queue.pop(0)
                for k in range(4):
                    i = 4 * self.c + k
                    if jq > i:
                        continue
                    # start=True clears has_written for the WHOLE bank, so
                    # only the very first matmul into this oT bank may set
                    # it; later first-writes hit cleared bits and overwrite.
                    nc.tensor.matmul(
                        self.oT[:, k, 0 : D + 1], lhsT=pTh[:, k * P : (k + 1) * P],
                        rhs=hsbs[self.b][:, jq, :],
                        start=(jq == 0 and k == 0), stop=(jq == i),
                        skip_group_check=True,
                    )

        def pair(self, j):
            """Steps j, j+1 into one 2-bank tile; ONE exp covers both."""
            b, c = self.b, self.c
            p2 = ps_ring.tile([P, 2 * CH], f32, tag="ring", name=f"p2_{b}_{c}_{j}")
            toffs = []
            for half, jj in enumerate((j, j + 1)):
                r = jj - 4 * c
                diag = r >= 0
                toff = 0 if not diag else P * r
                base = half * CH
                nc.tensor.matmul(
                    p2[:, base + toff : base + CH],
                    lhsT=hcs[b][:, jj * P : (jj + 1) * P],
                    rhs=qcs[b][:, c * CH + toff : (c + 1) * CH],
                    start=True, stop=not diag, skip_group_check=True,
                )
                if diag:
                    nc.tensor.matmul(
                        p2[:, base + toff : base + toff + P], lhsT=identh,
                        rhs=lmask, start=False, stop=True, skip_group_check=True,
                    )
                toffs.append(toff)
            pT = ppool.tile([P, 2 * CH], bf16, tag="pT")
            # One exp over both halves; the second half's [CH : CH+toff1)
            # may exponentiate stale PSUM, but AV never reads those columns.
            nc.scalar.activation(
                pT[:, toffs[0] : 2 * CH], p2[:, toffs[0] : 2 * CH],
                mybir.ActivationFunctionType.Exp,
            )
            self.av_queue.append((j, pT[:, 0:CH]))
            self.av_queue.append((j + 1, pT[:, CH : 2 * CH]))
            self.flush_av(2)

        def emitters(self):
            ems = [lambda jj=j: self.pair(jj) for j in range(0, self.nsteps, 2)]
            ems.append(self.finish)
            return ems

        def finish(self):
            self.flush_av(0)
            # Finalize: strided reciprocal of the 4 l columns (one DVE op),
            # then per-tile scale alternating DVE/ACT, one output DMA.
            oT, osb = self.oT, self.osb
            rl = spool.tile([P, 4], f32, tag="rl")
            nc.vector.reciprocal(rl, oT[:, :, D : D + 1])
            for k in range(4):
                if (k % 2 == 0) != self.scale_act:
                    nc.vector.tensor_scalar_mul(
                        osb[:, k, :], oT[:, k, 0:D], rl[:, k : k + 1]
                    )
                else:
                    nc.scalar.activation(
                        osb[:, k, :], oT[:, k, 0:D],
                        mybir.ActivationFunctionType.Copy, scale=rl[:, k : k + 1],
                    )
            nc.sync.dma_start(
                out=out_ext[self.b, self.c * CH : (self.c + 1) * CH, :].rearrange(
                    "(j p) d -> p j d", p=P
                ),
                in_=osb,
            )

    # ---- interleaved schedule ----
    # Ramp: batch-0 chunk 0/1 pre + stats(0,0); then zip rounds pairing
    # each p2 chunk with the NEXT stats group so the ACT exp stream and
    # DVE reduce stream run concurrently; pre blobs ride along.
    emit_pre(0, 0, first=True)
    for em in stats_emitters(0, 0):
        em()
    emit_pre(0, 1)

    def p2_ems(b, c, scale_act=False):
        return P2Chunk(b, c, scale_act).emitters()

    for em in stats_emitters(0, 1):
        em()
    rounds = [
        (p2_ems(0, 0, scale_act=True), stats_emitters(0, 2),
         [lambda: emit_pre(0, 2), lambda: emit_pre(1, 0)]),
        (p2_ems(0, 1, scale_act=True), stats_emitters(0, 3),
         [lambda: emit_pre(0, 3), lambda: emit_pre(1, 1),
          lambda: emit_pre(1, 2)]),
        (p2_ems(0, 2), stats_emitters(1, 3), [lambda: emit_pre(1, 3)]),
        (p2_ems(0, 3), stats_emitters(1, 2), []),
        (p2_ems(1, 3), stats_emitters(1, 1), []),
        (p2_ems(1, 2), stats_emitters(1, 0), []),
        (p2_ems(1, 1), [], []),
        (p2_ems(1, 0), [], []),
    ]
    for p2l, stl, prel in rounds:
        # pre blobs lead (their hc/qc feed the interleaved stats windows of
        # LATER rounds), then p2 pairs proportionally merged with stats.
        for em in prel:
            em()
        for em in _bresenham_merge(p2l, stl):
            em()


_cache = {}


def _get_nc():
    if "nc" not in _cache:
        nc = bacc.Bacc(
            "TRN2", target_bir_lowering=False, debug=False, num_devices=H
        )
        h_ext = nc.dram_tensor("h", [B, T, D], f32, kind="ExternalInput").ap()
        A_ext = nc.dram_tensor("A", [D, D], f32, kind="ExternalInput").ap()
        out_ext = nc.dram_tensor("out", [B, T, D], f32, kind="ExternalOutput").ap()
        with tile.TileContext(nc) as tc:
            with ExitStack() as ctx:
                _build(ctx, tc, h_ext, A_ext, out_ext)
        nc.compile()
        _cache["nc"] = nc
    return _cache["nc"]


def run(h, A, **kw):
    """Run on hardware; returns (full output [B,T,H*D], BassKernelResults)."""
    nc = _get_nc()
    h = np.ascontiguousarray(h, dtype=np.float32)
    A = np.ascontiguousarray(A, dtype=np.float32)
    in_maps = [{"h": h, "A": np.ascontiguousarray(A[i])} for i in range(H)]
    res = run_bass_kernel_spmd(nc, in_maps, core_ids=list(range(H)), **kw)
    out = np.concatenate([res.results[i]["out"] for i in range(H)], axis=-1)
    return out, res


def kernel(h, A):
    out, _ = run(h, A)
    return out


# revision 9
# speedup vs baseline: 1.2621x; 1.0120x over previous
"""Causal self-attention (per-head A projections) on 8 TRN2 NeuronCores.

Shapes: h [B=2, T=2048, d=64] f32, A [H=8, d, d] f32.
  q = h @ A[i]; scores = q @ h^T (causal); out_i = softmax(scores) @ h.
Sharding: one head per core (embarrassingly parallel, no collectives).
Each core receives the full h and its own A[i]; host concatenates heads.

v4: fp16 score path (1 cyc/col on the PE; f32r measured ~3.3 cyc/col on
HW) with bf16 only after the exp (pT/AV need bf16's exponent range);
masks use -60000 (fp16-representable; exp -> 0).

The kernel is throughput-bound on the exp (ACT) and row-max reduce (DVE)
consumers, not the PE.  Two structural rules follow:

1. Wide consumer ops: stats window PAIRS and p2 step PAIRS share one
   2-bank PSUM tile, so ONE DVE reduce / ONE ACT exp covers 1024
   columns (the ~200-cycle per-op PSUM overhead halves).

2. Fine-grained interleave: each p2 pair is emitted back-to-back with a
   stats pair from a LATER unit (Bresenham-merged), so the ACT exp
   stream and the DVE reduce stream run concurrently instead of
   alternating unit-by-unit through the in-order PE queue.  Schedule:
   batch 0 chunks ascend while their successor stats interleave; batch 1
   descends (its stats run against batch 0's largest p2 chunks).

PSUM (8 banks): shared 3-deep ring of [128, 1024] 2-bank tiles
(pre/stats/p2 - a matmul's ~450ns completion latency + ~100ns semaphore
+ ~1.1us consumer would stall a shallower dedicated ring) + oT x2.

  PRE(b,c): PE-transpose h chunk (fp16, packed pairs in PSUM cells via a
      bitcast view) into bank 0 of a ring tile; DVE copies hc out as
      16-bit; qT = A-matmul into bank 1 (f32); ACT copies qc out.  Pool
      casts hs16 (fp16 transpose src); DVE casts hsb (bf16 AV rhs, 4x).
  S(b,g): stats row-max for t-tiles 4g..4g+3, window pairs packed
      against the bank boundary so one DVE reduce consumes each pair;
      -60000 upper-triangle via identity matmul on the causal boundary
      window; -m (fp16) lands in row 64 of qc via a [128,1] SBUF DMA.
  P(b,c): pass2 scoresT [s,t] step pairs (fp16, K=65 so the -m row folds
      the shift into the matmul), diagonal tiles at exact causal width
      with -60000 from an identity matmul; ONE ACT exp per pair -> pT
      bf16 (the second half's acausal head may exp garbage; those pT
      columns are never read by AV); AV accumulates natural-layout out
      tiles [t, d+1] in PSUM (ones column of hsb = denominator l).
      NOTE: matmul start=True clears has_written for the WHOLE PSUM
      bank, so only the first AV matmul into each oT bank sets it.
      Finalize: one strided DVE reciprocal of the 4 l columns + per-tile
      scale (DVE/ACT alternating), one output DMA per chunk.
"""

import os
import sys

for _p in ("/opt/trn_rl_repo",):
    if _p not in sys.path:
        sys.path.insert(0, _p)

import numpy as np
from contextlib import ExitStack

import concourse.bass as bass
import concourse.tile as tile
from concourse import bacc, mybir
from concourse.masks import make_identity
from concourse.bass_utils import run_bass_kernel_spmd

B, T, D, H = 2, 2048, 64, 8
P = 128                # square tile size (t and s)
NT = T // P            # 16 tiles along t/s
CH = 512               # chunk width (PSUM bank)
NCH = T // CH          # 4 chunks
NEG = -60000.0         # fp16-representable -inf stand-in (exp -> 0)

f32 = mybir.dt.float32
fp16 = mybir.dt.float16
bf16 = mybir.dt.bfloat16


def _stat_windows(i):
    """Column windows covering the causal region [0, 128*(i+1)) for t-tile i.

    First window is rem wide (s_end % 512); remaining windows are 512 wide,
    end-aligned so the last one lands exactly on the causal boundary (where
    the mask goes).
    """
    s_end = (i + 1) * P
    rem = s_end % CH
    wins = []
    start = 0
    if rem:
        wins.append((0, rem))
        start = rem
    wins.extend(
        (start + k * CH, start + (k + 1) * CH) for k in range((s_end - start) // CH)
    )
    return wins



def _pieces(lo, hi, piece=256):
    out = []
    x = lo
    while x < hi:
        y = min(x + piece, hi)
        out.append((x, y))
        x = y
    return out

def _bresenham_merge(a, b):
    """Proportionally interleave two emitter lists (a paced against b)."""
    out = []
    na, nb = len(a), len(b)
    if not na:
        return list(b)
    if not nb:
        return list(a)
    ia = ib = 0
    err = 0
    while ia < na or ib < nb:
        if ib >= nb or (ia < na and err * nb < na * (2 * ib + 1) - nb):
            out.append(a[ia]); ia += 1; err += 2 * nb
        else:
            out.append(b[ib]); ib += 1
    return out


def _build(ctx: ExitStack, tc: "tile.TileContext", h_ext, A_ext, out_ext):
    nc = tc.nc

    consts = ctx.enter_context(tc.tile_pool(name="consts", bufs=1))
    hpool = ctx.enter_context(tc.tile_pool(name="hpool", bufs=2))
    qpool = ctx.enter_context(tc.tile_pool(name="qpool", bufs=2))
    ppool = ctx.enter_context(tc.tile_pool(name="ppool", bufs=4))
    spool = ctx.enter_context(tc.tile_pool(name="spool", bufs=4))
    opool = ctx.enter_context(tc.tile_pool(name="opool", bufs=2))
    # PSUM (8 banks): shared 3-deep ring of 2-bank tiles + oT x2.
    ps_ring = ctx.enter_context(tc.tile_pool(name="ps_ring", bufs=3, space="PSUM"))
    ps_out = ctx.enter_context(tc.tile_pool(name="ps_out", bufs=2, space="PSUM"))

    # ---- constants ----
    identh = consts.tile([P, P], fp16)
    make_identity(nc, identh)

    # umask[t, s] = NEG where s > t else 0 (stats-pass diagonal tile).
    umask = consts.tile([P, P], fp16)
    nc.gpsimd.memset(umask, 0.0)
    nc.gpsimd.affine_select(
        out=umask, in_=umask, compare_op=mybir.AluOpType.is_ge, fill=NEG,
        base=0, channel_multiplier=1, pattern=[[-1, P]],
    )
    # lmask[s, t] = NEG where t < s else 0 (pass-2 diagonal tile, scoresT).
    lmask = consts.tile([P, P], fp16)
    nc.gpsimd.memset(lmask, 0.0)
    nc.gpsimd.affine_select(
        out=lmask, in_=lmask, compare_op=mybir.AluOpType.is_ge, fill=NEG,
        base=0, channel_multiplier=-1, pattern=[[1, P]],
    )
    # A for this core's head: [d, e] natural layout (d on partitions), fp16.
    Asb32 = consts.tile([D, D], f32)
    nc.sync.dma_start(out=Asb32, in_=A_ext)
    Asb = consts.tile([D, D], fp16)
    nc.scalar.copy(Asb, Asb32)

    # HAM warmup: ~64 constant K=128 matmuls (~5-7us) while the input DMAs
    # land.  The PE clock gate defaults to 4/8 (1.2 GHz) and only a
    # sustained ~3.4us busy window at high array activity releases it;
    # without this the whole kernel streams matmuls at half clock.
    wtile = ps_ring.tile([P, 2 * CH], f32, tag="ring", name="warm")
    for r in range(64):
        nc.tensor.matmul(
            wtile[:, 0:P], lhsT=identh, rhs=umask,
            start=True, stop=True, skip_group_check=True,
        )

    # ---- bulk input loads (both batches up front) ----
    hs32s, hs16s, hsbs, hcs, qcs = [], [], [], [], []
    for b in range(B):
        hs32 = hpool.tile([P, NT, D + 1], f32, tag="hs32", name=f"hs32_{b}")
        h_re = h_ext[b].rearrange("(j p) d -> p j d", p=P)
        if b == 0:
            # fine-grained first-chunk loads so the first transpose starts
            # as early as possible (shorter pipeline ramp)
            for j in range(4):
                nc.sync.dma_start(out=hs32[:, j, 0:D], in_=h_re[:, j, :])
            nc.sync.dma_start(out=hs32[:, 4:NT, 0:D], in_=h_re[:, 4:NT, :])
        else:
            for c in range(NCH):
                nc.sync.dma_start(
                    out=hs32[:, 4 * c : 4 * c + 4, 0:D],
                    in_=h_re[:, 4 * c : 4 * c + 4, :],
                )
        nc.gpsimd.memset(hs32[:, :, D : D + 1], 1.0)
        hs32s.append(hs32)
        hs16s.append(hpool.tile([P, NT, D + 1], fp16, tag="hs16", name=f"hs16_{b}"))
        hsbs.append(hpool.tile([P, NT, D + 1], bf16, tag="hsb", name=f"hsb_{b}"))
        hcs.append(hpool.tile([D + 1, T], fp16, tag="hc", name=f"hc_{b}"))
        qcs.append(qpool.tile([D + 1, T], fp16, tag="qc", name=f"qc_{b}"))

    def emit_pre(b, c, first=False):
        """Transpose h chunk + qT chunk through one 2-bank ring PSUM tile."""
        lo = c * CH
        sl = (slice(None), slice(4 * c, 4 * c + 4), slice(None))
        # fp16 cast for the transpose source: DVE for the ramp-critical first
        # chunk, Pool otherwise; bf16 AV rhs derived on DVE at 4x.
        if first:
            nc.vector.tensor_copy(hs16s[b][sl], hs32s[b][sl])
            nc.vector.tensor_copy(hsbs[b][sl], hs16s[b][sl])
        else:
            nc.gpsimd.tensor_copy(hs16s[b][sl], hs32s[b][sl])
            nc.gpsimd.tensor_copy(hsbs[b][sl], hs16s[b][sl])

        stile = ps_ring.tile([P, 2 * CH], f32, tag="ring", name=f"pre_{b}_{c}")
        pt = stile[0 : D + 1, 0 : CH // 2].bitcast(fp16)
        for r in range(4):
            j = 4 * c + r
            nc.tensor.transpose(pt[:, r * P : (r + 1) * P], hs16s[b][:, j, :], identh)
        hc = hcs[b][:, lo : lo + CH]
        nc.vector.tensor_copy(hc, pt)

        pq = stile[0:D, CH : 2 * CH]
        for (x, y) in _pieces(0, CH):
            nc.tensor.matmul(pq[:, x:y], lhsT=Asb, rhs=hc[0:D, x:y],
                             start=x == 0, stop=y == CH, skip_group_check=True)
        nc.scalar.copy(qcs[b][0:D, lo : lo + CH], pq)

    def stats_emitters(b, g):
        """Per-PAIR emitters for the row-max of t-tiles 4g..4g+3."""
        ems = []
        for i in range(4 * g, 4 * g + 4):
            wins = _stat_windows(i)
            npair = (len(wins) + 1) // 2
            state = {}

            def make(i, p0, wins, npair, state):
                def em():
                    s_end = (i + 1) * P
                    if p0 == 0:
                        state["mxp"] = spool.tile([P, 2], f32, tag="mxp", name=f"mxp_{b}_{i}")
                        state["negm"] = spool.tile([P, 1], fp16, tag="negm", name=f"negm_{b}_{i}")
                        state["nred"] = 0
                    pair = wins[p0 : p0 + 2]
                    flat = ps_ring.tile([P, 2 * CH], f32, tag="ring",
                                        name=f"st_{b}_{i}_{p0}")
                    if len(pair) == 2:
                        (lo0, hi0), (lo1, hi1) = pair
                        w0, w1 = hi0 - lo0, hi1 - lo1
                        spans = [(lo0, hi0, CH - w0, CH), (lo1, hi1, CH, CH + w1)]
                    else:
                        (lo0, hi0) = pair[0]
                        spans = [(lo0, hi0, 0, hi0 - lo0)]
                    lhs_q = qcs[b][0:D, i * P : (i + 1) * P]
                    for lo, hi, a, bnd in spans:
                        pcs = _pieces(0, hi - lo)
                        for (x, y) in pcs:
                            nc.tensor.matmul(
                                flat[:, a + x : a + y], lhsT=lhs_q,
                                rhs=hcs[b][0:D, lo + x : lo + y],
                                start=x == 0,
                                stop=(y == hi - lo) and not hi == s_end,
                                skip_group_check=True,
                            )
                        if hi == s_end:
                            nc.tensor.matmul(
                                flat[:, bnd - P : bnd], lhsT=identh, rhs=umask,
                                start=False, stop=True, skip_group_check=True,
                            )
                    a0, a1 = spans[0][2], spans[-1][3]
                    if npair == 1:
                        nc.vector.reduce_max(
                            state["negm"], flat[:, a0:a1],
                            axis=mybir.AxisListType.X, negate=True,
                        )
                    else:
                        nc.vector.reduce_max(
                            state["mxp"][:, state["nred"] : state["nred"] + 1],
                            flat[:, a0:a1], axis=mybir.AxisListType.X,
                        )
                    state["nred"] += 1
                    if p0 + 2 >= len(wins):
                        if npair > 1:
                            nc.vector.reduce_max(
                                state["negm"], state["mxp"][:, 0 : state["nred"]],
                                axis=mybir.AxisListType.X, negate=True,
                            )
                        # Partition-column -> free-row reshape via tiny DMA.
                        nc.sync.dma_start(
                            out=qcs[b][D : D + 1, i * P : (i + 1) * P],
                            in_=state["negm"],
                        )
                return em

            for p0 in range(0, len(wins), 2):
                ems.append(make(i, p0, wins, npair, state))
        return ems

    class P2Chunk:
        """Pass2 scoresT + exp + natural-layout AV + finalize for one chunk."""

        def __init__(self, b, c, scale_act=False):
            self.b, self.c = b, c
            self.scale_act = scale_act
            self.oT = ps_out.tile([P, 4, D + 8], f32, tag="oT", name=f"oT_{b}_{c}")
            self.osb = opool.tile([P, 4, D], f32, tag="osb", name=f"osb_{b}_{c}")
            self.av_queue = []
            self.nsteps = 4 * c + 4

        def flush_av(self, limit):
            while len(self.av_queue) > limit:
                jq, pTh = self.av_queue.pop(0)
                for k in range(4):
                    i = 4 * self.c + k
                    if jq > i:
                        continue
                    # start=True clears has_written for the WHOLE bank, so
                    # only the very first matmul into this oT bank may set
                    # it; later first-writes hit cleared bits and overwrite.
                    nc.tensor.matmul(
                        self.oT[:, k, 0 : D + 1], lhsT=pTh[:, k * P : (k + 1) * P],
                        rhs=hsbs[self.b][:, jq, :],
                        start=(jq == 0 and k == 0), stop=(jq == i),
                        skip_group_check=True,
                    )

        def pair(self, j):
            """Steps j, j+1 into one 2-bank tile; ONE exp covers both."""
            b, c = self.b, self.c
            p2 = ps_ring.tile([P, 2 * CH], f32, tag="ring", name=f"p2_{b}_{c}_{j}")
            toffs = []
            for half, jj in enumerate((j, j + 1)):
                r = jj - 4 * c
                diag = r >= 0
                toff = 0 if not diag else P * r
                base = half * CH
                pcs = _pieces(toff, CH)
                for (x, y) in pcs:
                    nc.tensor.matmul(
                        p2[:, base + x : base + y],
                        lhsT=hcs[b][:, jj * P : (jj + 1) * P],
                        rhs=qcs[b][:, c * CH + x : c * CH + y],
                        start=x == toff, stop=(y == CH) and not diag,
                        skip_group_check=True,
                    )
                if diag:
                    nc.tensor.matmul(
                        p2[:, base + toff : base + toff + P], lhsT=identh,
                        rhs=lmask, start=False, stop=True, skip_group_check=True,
                    )
                toffs.append(toff)
            pT = ppool.tile([P, 2 * CH], bf16, tag="pT")
            # One exp over both halves; the second half's [CH : CH+toff1)
            # may exponentiate stale PSUM, but AV never reads those columns.
            nc.scalar.activation(
                pT[:, toffs[0] : 2 * CH], p2[:, toffs[0] : 2 * CH],
                mybir.ActivationFunctionType.Exp,
            )
            self.av_queue.append((j, pT[:, 0:CH]))
            self.av_queue.append((j + 1, pT[:, CH : 2 * CH]))
            self.flush_av(2)

        def emitters(self):
            ems = [lambda jj=j: self.pair(jj) for j in range(0, self.nsteps, 2)]
            ems.append(self.finish)
            return ems

        def finish(self):
            self.flush_av(0)
            # Finalize: strided reciprocal of the 4 l columns (one DVE op),
            # then per-tile scale alternating DVE/ACT, one output DMA.
            oT, osb = self.oT, self.osb
            rl = spool.tile([P, 4], f32, tag="rl")
            nc.vector.reciprocal(rl, oT[:, :, D : D + 1])
            for k in range(4):
                if (k % 2 == 0) != self.scale_act:
                    nc.vector.tensor_scalar_mul(
                        osb[:, k, :], oT[:, k, 0:D], rl[:, k : k + 1]
                    )
                else:
                    nc.scalar.activation(
                        osb[:, k, :], oT[:, k, 0:D],
                        mybir.ActivationFunctionType.Copy, scale=rl[:, k : k + 1],
                    )
            nc.sync.dma_start(
                out=out_ext[self.b, self.c * CH : (self.c + 1) * CH, :].rearrange(
                    "(j p) d -> p j d", p=P
                ),
                in_=osb,
            )

    # ---- interleaved schedule ----
    # Ramp: batch-0 chunk 0/1 pre + stats(0,0); then zip rounds pairing
    # each p2 chunk with the NEXT stats group so the ACT exp stream and
    # DVE reduce stream run concurrently; pre blobs ride along.
    emit_pre(0, 0, first=True)
    for em in stats_emitters(0, 0):
        em()
    emit_pre(0, 1)

    def p2_ems(b, c, scale_act=False):
        return P2Chunk(b, c, scale_act).emitters()

    for em in stats_emitters(0, 1):
        em()
    rounds = [
        (p2_ems(0, 0, scale_act=True), stats_emitters(0, 2),
         [lambda: emit_pre(0, 2), lambda: emit_pre(1, 0)]),
        (p2_ems(0, 1, scale_act=True), stats_emitters(0, 3),
         [lambda: emit_pre(0, 3), lambda: emit_pre(1, 1),
          lambda: emit_pre(1, 2)]),
        (p2_ems(0, 2), stats_emitters(1, 3), [lambda: emit_pre(1, 3)]),
        (p2_ems(0, 3), stats_emitters(1, 2), []),
        (p2_ems(1, 3), stats_emitters(1, 1), []),
        (p2_ems(1, 2), stats_emitters(1, 0), []),
        (p2_ems(1, 1), [], []),
        (p2_ems(1, 0), [], []),
    ]
    for p2l, stl, prel in rounds:
        # pre blobs lead (their hc/qc feed the interleaved stats windows of
        # LATER rounds), then p2 pairs proportionally merged with stats.
        for em in prel:
            em()
        for em in _bresenham_merge(p2l, stl):
            em()


_cache = {}


def _get_nc():
    if "nc" not in _cache:
        nc = bacc.Bacc(
            "TRN2", target_bir_lowering=False, debug=False, num_devices=H
        )
        h_ext = nc.dram_tensor("h", [B, T, D], f32, kind="ExternalInput").ap()
        A_ext = nc.dram_tensor("A", [D, D], f32, kind="ExternalInput").ap()
        out_ext = nc.dram_tensor("out", [B, T, D], f32, kind="ExternalOutput").ap()
        with tile.TileContext(nc) as tc:
            with ExitStack() as ctx:
                _build(ctx, tc, h_ext, A_ext, out_ext)
        nc.compile()
        _cache["nc"] = nc
    return _cache["nc"]


def run(h, A, **kw):
    """Run on hardware; returns (full output [B,T,H*D], BassKernelResults)."""
    nc = _get_nc()
    h = np.ascontiguousarray(h, dtype=np.float32)
    A = np.ascontiguousarray(A, dtype=np.float32)
    in_maps = [{"h": h, "A": np.ascontiguousarray(A[i])} for i in range(H)]
    res = run_bass_kernel_spmd(nc, in_maps, core_ids=list(range(H)), **kw)
    out = np.concatenate([res.results[i]["out"] for i in range(H)], axis=-1)
    return out, res


def kernel(h, A):
    out, _ = run(h, A)
    return out
